# revision 15
# baseline (speedup 1.0000x reference)
"""BitNet attention block on 8 TRN2 NeuronCores (tensor-parallel over heads).

Self-contained: kernel(**inputs) takes full inputs, shards internally,
runs one SPMD Bass program on cores 0-7, reassembles the full output.

Sharding: core c owns Q heads [4c,4c+4), KV head c, o_proj output dims
[512c, 512c+512). Attention is fully local per core. Cross-core comms:
two tiny AllReduces for the BitNet absmean gammas and four chunked
bf16 AllGathers (one per (batch, 512-token q-chunk)) of the attention
output so each core can compute its o_proj output-dim shard.

Key structure (vs a naive phase-serial version):
- Weights are DMA'd straight into their SBUF matmul storage once,
  abs-reduced from there, and ternarized IN PLACE to unscaled {-1,0,+1}
  after the gamma AllReduce; the first projection matmuls chase the
  per-tile quantization so the preamble is just DMA + AllReduce.
- All gamma scale factors are folded downstream instead of into the
  weights: gamma_q*gamma_k/sqrt(HD) into the softmax exp scale,
  gamma_v into the 1/z correction, gamma_o into the o_proj PSUM copy.
- Wo is abs-reduced in the preamble, then re-streamed and ternarized to
  bf16 during phase 1 (gpsimd+vector engines, which are idle then), so
  attention/phase 3 never wait on weight prep and Wo never bounces
  through DRAM.
- The attention output AllGathers are chunked per (batch, q-chunk) in
  bf16 and overlap attention + o_proj compute.
- o_proj runs in bf16 (ternary Wo is exact in bf16; activations lose
  ~0.4% which is far inside the 2e-2 gate).
"""
import os
import sys
sys.path.insert(0, "/opt/trn_rl_repo")
import numpy as np

B, S, H = 2, 1024, 4096
NH, NKV, HD = 32, 8, 128
NCORES = 8
T = B * S
QH = NH // NCORES          # 4 q-heads per core
MSH = H // NCORES          # 512 o_proj out-dims per core
THETA = 10000.0
C_MAGIC = 12582912.0       # 1.5 * 2**23: (x + C) - C == round-half-even(x)
TWO_PI = 6.283185307179586
NKT = H // 128             # 32 contraction tiles
NTC = T // 512             # 4 token chunks
SKT = S // 128             # 8 score k-tiles per batch
SQC = S // 512             # 2 q-chunks per batch

_cache = {}
last_exec_time_ns = None


def _classify_mask(mask):
    """Per (b, kt, qc) [128k x 512q] block: 0 zero, 1 masked-out, 2 general.

    A block whose mask values are all <= -1e4 contributes exactly 0 to the
    fp32 softmax (exp underflows to 0.0), so it is skipped entirely.
    """
    status = np.empty((B, SKT, SQC), dtype=np.int8)
    index = {}
    packed = []
    for b in range(B):
        mb = np.asarray(mask[b, 0], dtype=np.float32)   # (q, k)
        for kt in range(SKT):
            for qc in range(SQC):
                blk = mb[qc * 512:(qc + 1) * 512, kt * 128:(kt + 1) * 128]
                if not blk.any():
                    status[b, kt, qc] = 0
                elif (blk <= -1e4).all():
                    status[b, kt, qc] = 1
                else:
                    status[b, kt, qc] = 2
                    index[(b, kt, qc)] = len(packed)
                    packed.append(np.ascontiguousarray(blk.T))  # (128k, 512q)
    if packed:
        packed_arr = np.concatenate(packed, axis=0).astype(np.float32)
    else:
        packed_arr = np.zeros((128, 512), dtype=np.float32)
    return status, index, packed_arr


def _cody_consts():
    c1 = float(np.float32(6.28125))
    r = np.float64(TWO_PI) - c1
    c2 = float(np.float32(r - np.remainder(r, 2.0 ** -24)))
    c3 = float(np.float32(np.float64(TWO_PI) - c1 - float(c2)))
    return c1, c2, c3


def _build(status, index, n_packed):
    from concourse import bacc, tile, mybir, bass_isa

    F32 = mybir.dt.float32
    F32R = mybir.dt.float32r
    BF16 = mybir.dt.bfloat16
    ACTF = mybir.ActivationFunctionType
    ALU = mybir.AluOpType
    X = mybir.AxisListType.X
    RG = [list(range(NCORES))]
    c1, c2, c3 = _cody_consts()
    ISQ = float(1.0 / np.sqrt(HD))

    nc = bacc.Bacc("TRN2", target_bir_lowering=False, debug=False,
                   num_devices=NCORES)

    xT = nc.dram_tensor("xT", [H, T], F32, kind="ExternalInput")
    wqT = nc.dram_tensor("wqT", [H, QH * HD], F32, kind="ExternalInput")
    wkT = nc.dram_tensor("wkT", [H, HD], F32, kind="ExternalInput")
    wvT = nc.dram_tensor("wvT", [H, HD], F32, kind="ExternalInput")
    woT = nc.dram_tensor("woT", [H, MSH], F32, kind="ExternalInput")
    maskP = nc.dram_tensor("maskP", [n_packed * 128, 512], F32,
                           kind="ExternalInput")
    pos = nc.dram_tensor("pos", [1, T], F32, kind="ExternalInput")
    outN = nc.dram_tensor("outN", [T, MSH], F32, kind="ExternalOutput")

    idn_c = nc.inline_tensor(np.eye(128, dtype=np.float32), name="idn_c")
    onesk_c = nc.inline_tensor(np.ones((128, 1), np.float32), name="onesk_c")
    onesm_c = nc.inline_tensor(np.ones((1, 128), np.float32), name="onesm_c")
    invf_np = (1.0 / THETA ** (np.arange(0, HD, 2, dtype=np.float32) / HD))
    invf_np = np.concatenate([invf_np, invf_np]).reshape(HD, 1)
    invf_c = nc.inline_tensor(invf_np.astype(np.float32), name="invf_c")

    NQ = float(NH * HD * H)
    NK = float(NKV * HD * H)
    NO = float(H * NH * HD)

    with tile.TileContext(nc) as tc, \
         nc.allow_low_precision(reason="float32r kernel"):
        with tc.tile_pool(name="cpool", bufs=1) as cpool, \
             tc.tile_pool(name="dbounce", bufs=1, space="DRAM") as dbounce:
            # DRAM bounce tiles (tracked by Tile for collective deps)
            arq_in = dbounce.tile([1, 4], F32, name="arq_in")
            arq_out = dbounce.tile([1, 4], F32, name="arq_out",
                                   addr_space="Shared")
            ark_in = dbounce.tile([1, 4], F32, name="ark_in")
            ark_out = dbounce.tile([1, 4], F32, name="ark_out",
                                   addr_space="Shared")
            agin = [dbounce.tile([QH * HD, 512], BF16, name=f"agin{ch}")
                    for ch in range(NTC)]
            agout = [dbounce.tile([H, 512], BF16, name=f"agout{ch}",
                                  addr_space="Shared") for ch in range(NTC)]
            wo_dram = dbounce.tile([H, MSH], BF16, name="wo_dram")

            idn = cpool.tile([128, 128], F32R, name="idn")
            nc.sync.dma_start(out=idn[:], in_=idn_c[:, :].bitcast(F32R))
            oneskr = cpool.tile([128, 1], F32R, name="oneskr")
            nc.sync.dma_start(out=oneskr[:], in_=onesk_c[:, :].bitcast(F32R))
            onesk = cpool.tile([128, 1], F32, name="onesk")
            nc.sync.dma_start(out=onesk[:], in_=onesk_c[:, :])
            onesmr = cpool.tile([1, 128], F32R, name="onesmr")
            nc.sync.dma_start(out=onesmr[:], in_=onesm_c[:, :].bitcast(F32R))
            onesm = cpool.tile([1, 128], F32, name="onesm")
            nc.sync.dma_start(out=onesm[:], in_=onesm_c[:, :])
            invf = cpool.tile([128, 1], F32, name="invf")
            nc.sync.dma_start(out=invf[:], in_=invf_c[:, :])
            cmag = cpool.tile([128, 1], F32, name="cmag")
            nc.vector.memset(cmag[:], C_MAGIC)
            bsc = cpool.tile([128, 8], F32, name="bsc")
            grow = cpool.tile([1, 8], F32, name="grow")

            # long-lived storage: qkv outputs, wo ternary, raw weights
            qkvp = tc.alloc_tile_pool(name="qkv", bufs=1)
            qT_sb = [qkvp.tile([128, T], F32R, name=f"qT{h}")
                     for h in range(QH)]
            kT_sb = qkvp.tile([128, T], F32R, name="kT_sb")
            vT_sb = qkvp.tile([128, T], F32R, name="vT_sb")
            wbig = tc.alloc_tile_pool(name="wbig", bufs=1)
            wq_sb = wbig.tile([128, NKT * 512], F32R, name="wq_sb")
            wk_sb = wbig.tile([128, NKT * 128], F32R, name="wk_sb")
            wv_sb = wbig.tile([128, NKT * 128], F32R, name="wv_sb")

            wspecs = [(wqT, 512, wq_sb), (wkT, 128, wk_sb), (wvT, 128, wv_sb)]

            def w_f32(wi, i):
                wt, fw, dst = wspecs[wi]
                return dst[:, i * fw:(i + 1) * fw].bitcast(F32)

            with tc.tile_pool(name="gacc", bufs=1) as gacc:
                # ---- load weights into final storage + per-tile |.| sums
                g4 = gacc.tile([128, 4], F32, name="g4")
                accs = [gacc.tile([128, NKT], F32, name=f"acc{wi}")
                        for wi in range(4)]
                for i in range(NKT):
                    for wi in range(3):
                        wt, fw, dst = wspecs[wi]
                        nc.sync.dma_start(
                            out=dst[:, i * fw:(i + 1) * fw],
                            in_=wt[i * 128:(i + 1) * 128, :].bitcast(F32R))
                        nc.vector.tensor_reduce(
                            accs[wi][:, i:i + 1], w_f32(wi, i), X, ALU.add,
                            apply_absolute_value=True)
                for wi in range(3):
                    nc.vector.tensor_reduce(g4[:, wi:wi + 1], accs[wi][:], X,
                                            ALU.add)
                # cross-partition totals on gpsimd (no PSUM needed)
                gsum = gacc.tile([128, 4], F32, name="gsum")
                nc.gpsimd.partition_all_reduce(gsum[:, 0:3], g4[:, 0:3], 128,
                                               bass_isa.ReduceOp.add)
                gq_sb = gacc.tile([1, 4], F32, name="gq_sb")
                nc.vector.memset(gq_sb[:], 0.0)
                nc.vector.tensor_copy(gq_sb[:, 0:3], gsum[0:1, 0:3])
                nc.sync.dma_start(out=arq_in[:], in_=gq_sb[:])
                nc.gpsimd.collective_compute(
                    "AllReduce", ALU.add, replica_groups=RG,
                    ins=[arq_in[:].opt()], outs=[arq_out[:].opt()])
                arq_sb = gacc.tile([1, 4], F32, name="arq_sb")
                nc.sync.dma_start(out=arq_sb[:], in_=arq_out[:])

                # gammas: grow = [1/gq, 1/gk, 1/gv, gq*gk/sqrt(d), gv, ...]
                gtmp = gacc.tile([1, 8], F32, name="gtmp")
                nc.vector.tensor_scalar(gtmp[:, 0:1], arq_sb[:, 0:1],
                                        1.0 / NQ, 1e-5, ALU.mult, ALU.add)
                nc.vector.tensor_scalar(gtmp[:, 1:3], arq_sb[:, 1:3],
                                        1.0 / NK, 1e-5, ALU.mult, ALU.add)
                nc.vector.reciprocal(grow[:, 0:3], gtmp[:, 0:3])
                # sqk = gq * gk / sqrt(HD)
                nc.vector.tensor_mul(gtmp[:, 3:4], gtmp[:, 0:1], gtmp[:, 1:2])
                nc.vector.tensor_scalar(grow[:, 3:4], gtmp[:, 3:4], ISQ, None,
                                        ALU.mult)
                nc.vector.tensor_copy(grow[:, 4:5], gtmp[:, 2:3])  # gv
                nc.gpsimd.partition_broadcast(bsc[:, 0:5], grow[:, 0:5])
                BIQ, BIK, BIV, BSQK = (bsc[:, i:i + 1] for i in range(4))

                # ---- in-place ternary quant to unscaled {-1,0,+1}
                with tc.tile_pool(name="qscr", bufs=2) as qscr:
                    for i in range(NKT):
                        for wi in range(3):
                            wt, fw, dst = wspecs[wi]
                            src = w_f32(wi, i)
                            s = qscr.tile([128, fw], F32,
                                          name=f"qs_{wi}_{i}", tag=f"qs{fw}")
                            inv_ap = bsc[:, wi:wi + 1]
                            # t = w*(1/gamma) + C  (exact round-half-even)
                            nc.scalar.activation(s[:], src, ACTF.Identity,
                                                 bias=cmag[:], scale=inv_ap)
                            # t = min(t - C, 1)
                            nc.vector.tensor_scalar(s[:], s[:], C_MAGIC, 1.0,
                                                    ALU.subtract, ALU.min)
                            # w_t = max(t, -1)   (unscaled ternary)
                            eng = nc.vector if wi == 0 else nc.gpsimd
                            eng.tensor_scalar(dst[:, i * fw:(i + 1) * fw],
                                              s[:], -1.0, None, ALU.max)

                # ---- wo prepass: stream once for the abs-sum only
                with tc.tile_pool(name="wop", bufs=3) as wop:
                    for i in range(NKT):
                        slf = wop.tile([128, 512], F32, name=f"wo_{i}",
                                       tag="wo")
                        nc.scalar.dma_start(
                            out=slf[:], in_=woT[i * 128:(i + 1) * 128, :])
                        nc.vector.tensor_reduce(
                            accs[3][:, i:i + 1], slf[:], X, ALU.add,
                            apply_absolute_value=True)
                    nc.vector.tensor_reduce(g4[:, 3:4], accs[3][:], X,
                                            ALU.add)
                    nc.gpsimd.partition_all_reduce(
                        gsum[:, 3:4], g4[:, 3:4], 128, bass_isa.ReduceOp.add)
                    gk_sb = gacc.tile([1, 4], F32, name="gk_sb")
                    nc.vector.memset(gk_sb[:], 0.0)
                    nc.vector.tensor_copy(gk_sb[:, 0:1], gsum[0:1, 3:4])
                    nc.sync.dma_start(out=ark_in[:], in_=gk_sb[:])
                    nc.gpsimd.collective_compute(
                        "AllReduce", ALU.add, replica_groups=RG,
                        ins=[ark_in[:].opt()], outs=[ark_out[:].opt()])
                    ark_sb = gacc.tile([1, 4], F32, name="ark_sb")
                    nc.sync.dma_start(out=ark_sb[:], in_=ark_out[:])
                    nc.vector.tensor_scalar(gtmp[:, 4:5], ark_sb[:, 0:1],
                                            1.0 / NO, 1e-5, ALU.mult, ALU.add)
                    nc.vector.reciprocal(grow[:, 5:6], gtmp[:, 4:5])
                    nc.vector.tensor_copy(grow[:, 6:7], gtmp[:, 4:5])  # go
                    nc.gpsimd.partition_broadcast(bsc[:, 5:7], grow[:, 5:7])
                BIO, BGO = bsc[:, 5:6], bsc[:, 6:7]

            with tc.tile_pool(name="tab", bufs=1) as tab:
                cos_sb = tab.tile([128, T], F32, name="cos_sb")
                ss_sb = tab.tile([128, T], F32, name="ss_sb")
                # RoPE tables: Cody-Waite range reduction + Sin
                with tc.tile_pool(name="rtab", bufs=2) as rtab, \
                     tc.tile_pool(name="prt", bufs=2, space="PSUM") as prt:
                    for tcn in range(NTC):
                        cs = slice(tcn * 512, (tcn + 1) * 512)
                        pchunk = rtab.tile([1, 512], F32, name=f"pos{tcn}",
                                           tag="pos")
                        nc.sync.dma_start(out=pchunk[:], in_=pos[0:1, cs])
                        pf = prt.tile([128, 512], F32, name=f"pf{tcn}",
                                      tag="pf")
                        nc.tensor.matmul(pf[:], onesm[:], pchunk[:],
                                         start=True, stop=True)
                        f_sb = rtab.tile([128, 512], F32, name=f"f{tcn}",
                                         tag="f")
                        nc.scalar.activation(f_sb[:], pf[:], ACTF.Copy,
                                             scale=invf[:])
                        k_sb = rtab.tile([128, 512], F32, name=f"kk{tcn}",
                                         tag="kk")
                        nc.vector.tensor_scalar(k_sb[:], f_sb[:],
                                                1.0 / TWO_PI, C_MAGIC,
                                                ALU.mult, ALU.add)
                        nc.vector.tensor_scalar(k_sb[:], k_sb[:], C_MAGIC,
                                                None, ALU.subtract)
                        y_sb = rtab.tile([128, 512], F32, name=f"y{tcn}",
                                         tag="y")
                        nc.vector.scalar_tensor_tensor(
                            y_sb[:], k_sb[:], -c1, f_sb[:], ALU.mult, ALU.add)
                        nc.vector.scalar_tensor_tensor(
                            y_sb[:], k_sb[:], -c2, y_sb[:], ALU.mult, ALU.add)
                        nc.vector.scalar_tensor_tensor(
                            y_sb[:], k_sb[:], -c3, y_sb[:], ALU.mult, ALU.add)
                        nc.scalar.activation(ss_sb[0:64, cs], y_sb[0:64, :],
                                             ACTF.Sin, scale=-1.0)
                        nc.scalar.activation(ss_sb[64:128, cs],
                                             y_sb[64:128, :], ACTF.Sin)
                        yc = rtab.tile([128, 512], F32, name=f"yc{tcn}",
                                       tag="yc")
                        nc.vector.tensor_scalar(yc[:], y_sb[:],
                                                float(np.pi / 2), None,
                                                ALU.add)
                        m_sb = rtab.tile([128, 512], F32, name=f"mm{tcn}",
                                         tag="mm")
                        nc.vector.tensor_scalar(m_sb[:], yc[:],
                                                float(np.pi), None, ALU.is_gt)
                        nc.vector.scalar_tensor_tensor(
                            yc[:], m_sb[:], -TWO_PI, yc[:], ALU.mult, ALU.add)
                        nc.scalar.activation(cos_sb[:, cs], yc[:], ACTF.Sin)

                # ---- phase 1: QKV projections + RoPE, wo quant interleaved
                with tc.tile_pool(name="xin", bufs=4) as xin, \
                     tc.tile_pool(name="rope", bufs=2) as rope, \
                     tc.tile_pool(name="wo2", bufs=2) as wo2, \
                     tc.tile_pool(name="p1", bufs=8, space="PSUM") as p1:

                    def wo_quant(i):
                        src = wo2.tile([128, 512], F32, name=f"wr_{i}",
                                       tag="wr")
                        nc.scalar.dma_start(
                            out=src[:], in_=woT[i * 128:(i + 1) * 128, :])
                        s = wo2.tile([128, 512], F32, name=f"ws_{i}",
                                     tag="ws")
                        nc.gpsimd.tensor_scalar(s[:], src[:], BIO, C_MAGIC,
                                                ALU.mult, ALU.add)
                        nc.vector.tensor_scalar(s[:], s[:], C_MAGIC, 1.0,
                                                ALU.subtract, ALU.min)
                        wt = wo2.tile([128, 512], BF16, name=f"wt_{i}",
                                      tag="wt")
                        nc.vector.tensor_scalar(wt[:], s[:], -1.0, None,
                                                ALU.max)
                        nc.scalar.dma_start(
                            out=wo_dram[i * 128:(i + 1) * 128, :], in_=wt[:])

                    for tcn in range(NTC):
                        cs = slice(tcn * 512, (tcn + 1) * 512)
                        pq = [p1.tile([128, 512], F32, name=f"pq{tcn}_{h}",
                                      tag="p1") for h in range(QH)]
                        pk = p1.tile([128, 512], F32, name=f"pk{tcn}",
                                     tag="p1")
                        pv = p1.tile([128, 512], F32, name=f"pv{tcn}",
                                     tag="p1")
                        for kt in range(NKT):
                            xt = xin.tile([128, 512], F32R,
                                          name=f"x{tcn}_{kt}", tag="xt")
                            nc.sync.dma_start(
                                out=xt[:],
                                in_=xT[kt * 128:(kt + 1) * 128, cs]
                                .bitcast(F32R))
                            st, sp = (kt == 0), (kt == NKT - 1)
                            for h in range(QH):
                                nc.tensor.matmul(
                                    pq[h][:],
                                    wq_sb[:, kt * 512 + h * 128:
                                          kt * 512 + (h + 1) * 128],
                                    xt[:], start=st, stop=sp,
                                    skip_group_check=True)
                            nc.tensor.matmul(
                                pk[:], wk_sb[:, kt * 128:(kt + 1) * 128],
                                xt[:], start=st, stop=sp,
                                skip_group_check=True)
                            nc.tensor.matmul(
                                pv[:], wv_sb[:, kt * 128:(kt + 1) * 128],
                                xt[:], start=st, stop=sp,
                                skip_group_check=True)

                        def rope_apply(psrc, dst_ap, tg):
                            m1 = rope.tile([128, 512], F32, name=f"m1{tg}",
                                           tag="m1")
                            nc.vector.tensor_mul(m1[:], psrc[:], cos_sb[:, cs])
                            m2 = rope.tile([128, 512], F32, name=f"m2{tg}",
                                           tag="m2")
                            nc.vector.tensor_mul(m2[0:64, :], psrc[64:128, :],
                                                 ss_sb[0:64, cs])
                            nc.vector.tensor_mul(m2[64:128, :], psrc[0:64, :],
                                                 ss_sb[64:128, cs])
                            nc.vector.tensor_add(dst_ap, m1[:], m2[:])
                        for h in range(QH):
                            rope_apply(pq[h], qT_sb[h][:, cs], f"_{tcn}_{h}")
                        rope_apply(pk, kT_sb[:, cs], f"k_{tcn}")
                        nc.scalar.copy(vT_sb[:, cs], pv[:])
                        for i in range(tcn * 8, tcn * 8 + 8):
                            wo_quant(i)

            wbig.release()  # free wq/wk/wv SBUF before attention
            # prefetch quantized bf16 wo back into SBUF during attention
            w3 = tc.alloc_tile_pool(name="w3", bufs=1)
            wo_sb = w3.tile([128, NKT * MSH], BF16, name="wo_sb")
            for i in range(NKT):
                nc.sync.dma_start(out=wo_sb[:, i * MSH:(i + 1) * MSH],
                                  in_=wo_dram[i * 128:(i + 1) * 128, :])

            # ---- phase 2: attention (chunked bf16 AllGather per (b,qc)) ----
            with tc.tile_pool(name="vnatp", bufs=2) as vnatp, \
                 tc.tile_pool(name="epool", bufs=8) as epool, \
                 tc.tile_pool(name="mpool", bufs=1) as mpool, \
                 tc.tile_pool(name="aop", bufs=4) as aop, \
                 tc.tile_pool(name="zpool", bufs=2) as zpool, \
                 tc.tile_pool(name="ps_s", bufs=2, space="PSUM") as ps_s, \
                 tc.tile_pool(name="ps_o", bufs=2, space="PSUM") as ps_o, \
                 tc.tile_pool(name="ps_x", bufs=2, space="PSUM") as ps_x:
                for b in range(B):
                    boff = b * S
                    vnat = vnatp.tile([128, S], F32R, name=f"vnat{b}",
                                      tag="vnat")
                    for kt in range(SKT):
                        ptr = ps_x.tile([128, 128], F32R, name=f"ptr{b}_{kt}",
                                        tag="ptr")
                        nc.tensor.transpose(
                            ptr[:],
                            vT_sb[:, boff + kt * 128:boff + (kt + 1) * 128],
                            idn[:])
                        nc.scalar.copy(vnat[:, kt * 128:(kt + 1) * 128],
                                       ptr[:].bitcast(F32))
                    for qc in range(SQC):
                        ch = b * SQC + qc
                        # mask tiles shared across the 4 heads
                        mtiles = {}
                        for kt in range(SKT):
                            if status[b, kt, qc] == 2:
                                mi = index[(b, kt, qc)]
                                mt_ = mpool.tile([128, 512], F32,
                                                 name=f"mt{b}{qc}{kt}",
                                                 tag=f"mt{kt}")
                                nc.scalar.dma_start(
                                    out=mt_[:],
                                    in_=maskP[mi * 128:(mi + 1) * 128, :])
                                mtiles[kt] = mt_
                        kts = [kt for kt in range(SKT)
                               if status[b, kt, qc] != 1]
                        assert kts, "fully-masked softmax row"
                        for h in range(QH):
                            qsl = qT_sb[h][:, boff + qc * 512:
                                           boff + (qc + 1) * 512]
                            pz = ps_x.tile([1, 512], F32, name=f"pz{b}{h}{qc}",
                                           tag="pz")
                            po = ps_o.tile([128, 512], F32,
                                           name=f"po{b}{h}{qc}", tag="po")
                            for i, kt in enumerate(kts):
                                ps_ = ps_s.tile([128, 512], F32,
                                                name=f"s{b}{h}{qc}{kt}",
                                                tag="ps")
                                nc.tensor.matmul(
                                    ps_[:],
                                    kT_sb[:, boff + kt * 128:
                                          boff + (kt + 1) * 128],
                                    qsl, start=True, stop=True,
                                    skip_group_check=True)
                                e = epool.tile([128, 512], F32R,
                                               name=f"e{b}{h}{qc}{kt}",
                                               tag="e")
                                if kt in mtiles:
                                    # scores*sqk + mask, then exp
                                    nc.vector.scalar_tensor_tensor(
                                        ps_[:], ps_[:], BSQK, mtiles[kt][:],
                                        ALU.mult, ALU.add)
                                    nc.scalar.activation(e[:], ps_[:],
                                                         ACTF.Exp)
                                else:
                                    nc.scalar.activation(e[:], ps_[:],
                                                         ACTF.Exp, scale=BSQK)
                                fst = (i == 0)
                                lst = (i == len(kts) - 1)
                                nc.tensor.matmul(
                                    pz[:], oneskr[:], e[:], start=fst,
                                    stop=lst, skip_group_check=True)
                                nc.tensor.matmul(
                                    po[:],
                                    vnat[:, kt * 128:(kt + 1) * 128], e[:],
                                    start=fst, stop=lst,
                                    skip_group_check=True)
                            # zr = gamma_v / z, broadcast to 128 partitions
                            zr = zpool.tile([1, 512], F32, name=f"zr{b}{h}{qc}",
                                            tag="zr")
                            nc.vector.reciprocal(zr[:], pz[:])
                            nc.vector.tensor_scalar(zr[:], zr[:],
                                                    grow[0:1, 4:5], None,
                                                    ALU.mult)
                            zb = zpool.tile([128, 512], F32,
                                            name=f"zb{b}{h}{qc}", tag="zb")
                            nc.gpsimd.partition_broadcast(zb[:], zr[:])
                            ao = aop.tile([128, 512], BF16,
                                          name=f"ao{b}{h}{qc}", tag="ao")
                            nc.vector.tensor_mul(ao[:], po[:], zb[:])
                            nc.sync.dma_start(
                                out=agin[ch][h * 128:(h + 1) * 128, :],
                                in_=ao[:])
                        nc.gpsimd.collective_compute(
                            "AllGather", ALU.bypass, replica_groups=RG,
                            ins=[agin[ch][:].opt()],
                            outs=[agout[ch][:].opt()])

            # ---- phase 3: o_proj in bf16 (out in natural [token, m]) ----
            with tc.tile_pool(name="a3", bufs=1) as a3, \
                 tc.tile_pool(name="o3", bufs=2) as o3, \
                 tc.tile_pool(name="p3", bufs=2, space="PSUM") as p3:
                for ch in range(NTC):
                    ats = []
                    for kt in range(NKT):
                        at = a3.tile([128, 512], BF16, name=f"at{ch}_{kt}",
                                     tag=f"at{kt}")
                        nc.sync.dma_start(
                            out=at[:],
                            in_=agout[ch][kt * 128:(kt + 1) * 128, :])
                        ats.append(at)
                    for tt in range(4):
                        pout = p3.tile([128, 512], F32, name=f"po3_{ch}{tt}",
                                       tag="pout")
                        for kt in range(NKT):
                            nc.tensor.matmul(
                                pout[:],
                                ats[kt][:, tt * 128:(tt + 1) * 128],
                                wo_sb[:, kt * MSH:(kt + 1) * MSH],
                                start=(kt == 0), stop=(kt == NKT - 1),
                                skip_group_check=True)
                        osb = o3.tile([128, 512], F32, name=f"osb{ch}{tt}",
                                      tag="osb")
                        nc.scalar.activation(osb[:], pout[:], ACTF.Copy,
                                             scale=BGO)
                        nc.sync.dma_start(
                            out=outN[ch * 512 + tt * 128:
                                     ch * 512 + (tt + 1) * 128, :],
                            in_=osb[:])
            w3.release()
            qkvp.release()

    nc.compile()
    return nc


def kernel(hidden_states, Wq, Wk, Wv, Wo, attention_mask, position_ids):
    from concourse.bass_utils import run_bass_kernel_spmd
    from concourse.bass_interp import get_hw_module

    hs = np.ascontiguousarray(np.asarray(hidden_states, dtype=np.float32))
    Wq = np.asarray(Wq, dtype=np.float32)
    Wk = np.asarray(Wk, dtype=np.float32)
    Wv = np.asarray(Wv, dtype=np.float32)
    Wo = np.asarray(Wo, dtype=np.float32)
    mask = np.asarray(attention_mask, dtype=np.float32)
    posf = np.ascontiguousarray(
        np.asarray(position_ids).reshape(1, T).astype(np.float32))

    status, index, packed = _classify_mask(mask)
    n_packed = packed.shape[0] // 128

    key = (status.tobytes(), n_packed)
    if key not in _cache:
        nc = _build(status, index, n_packed)
        nc.m = get_hw_module(nc.m)
        _cache[key] = nc
    nc = _cache[key]

    xT = np.ascontiguousarray(hs.reshape(T, H).T)
    in_maps = []
    for c in range(NCORES):
        in_maps.append({
            "xT": xT,
            "wqT": np.ascontiguousarray(
                Wq[c * QH * HD:(c + 1) * QH * HD, :].T),
            "wkT": np.ascontiguousarray(Wk[c * HD:(c + 1) * HD, :].T),
            "wvT": np.ascontiguousarray(Wv[c * HD:(c + 1) * HD, :].T),
            "woT": np.ascontiguousarray(Wo[c * MSH:(c + 1) * MSH, :].T),
            "maskP": packed,
            "pos": posf,
        })
    res = run_bass_kernel_spmd(nc, in_maps, core_ids=list(range(NCORES)),
                               trace=bool(os.environ.get("BITNET_TRACE")))
    global last_exec_time_ns
    last_exec_time_ns = res.exec_time_ns
    out = np.concatenate(
        [res.results[c]["outN"] for c in range(NCORES)], axis=1)  # (T, H)
    return np.ascontiguousarray(out).reshape(B, S, H).astype(np.float32)


# revision 16
# speedup vs baseline: 1.1989x; 1.1989x over previous
"""BitNet attention block on 8 TRN2 NeuronCores (tensor-parallel over heads).

Self-contained: kernel(**inputs) takes full inputs, shards internally,
runs one SPMD Bass program on cores 0-7, reassembles the full output.

Sharding: core c owns Q heads [4c,4c+4), KV head c, o_proj output dims
[512c, 512c+512). Attention is fully local per core. Cross-core comms:
two tiny AllReduces for the BitNet absmean gammas and four chunked
bf16 AllGathers (one per (batch, 512-token q-chunk)) of the attention
output so each core can compute its o_proj output-dim shard.

Key structure (vs a naive phase-serial version):
- Weights are DMA'd straight into their SBUF matmul storage once
  (split across the SP and ACT DMA queues), abs-reduced from there, and
  ternarized IN PLACE to unscaled {-1,0,+1} right after the gamma
  AllReduce; the first projection matmuls chase the per-tile quant so
  the preamble is just DMA + AllReduce.
- All gamma scale factors are folded downstream instead of into the
  weights: gamma_q*gamma_k/sqrt(HD) into the softmax exp scale,
  gamma_v into the 1/z correction, gamma_o into the o_proj PSUM copy.
  The ternary weights are exact in every dtype, so o_proj runs in bf16.
- Wo is abs-reduced in the preamble, then re-streamed + ternarized to
  bf16 during phase 1 (engines are idle then), staged through DRAM, and
  prefetched into SBUF before o_proj needs it.
- GpSimd does only partition broadcasts/reductions and collective
  triggers (its elementwise path is ~8-15x slower than DVE).
- Engine FIFOs are in-order, so emission order is chosen to keep every
  queue's head runnable: rope tables first, gammas before quant, wo
  chains interleaved late in phase 1.
"""
import os
import sys
sys.path.insert(0, "/opt/trn_rl_repo")
import numpy as np

B, S, H = 2, 1024, 4096
NH, NKV, HD = 32, 8, 128
NCORES = 8
T = B * S
QH = NH // NCORES          # 4 q-heads per core
MSH = H // NCORES          # 512 o_proj out-dims per core
THETA = 10000.0
C_MAGIC = 12582912.0       # 1.5 * 2**23: (x + C) - C == round-half-even(x)
TWO_PI = 6.283185307179586
NKT = H // 128             # 32 contraction tiles
NTC = T // 512             # 4 token chunks
SKT = S // 128             # 8 score k-tiles per batch
SQC = S // 512             # 2 q-chunks per batch

_cache = {}
last_exec_time_ns = None


def _classify_mask(mask):
    """Per (b, kt, qc) [128k x 512q] block: 0 zero, 1 masked-out, 2 general.

    A block whose mask values are all <= -1e4 contributes exactly 0 to the
    fp32 softmax (exp underflows to 0.0), so it is skipped entirely.
    """
    status = np.empty((B, SKT, SQC), dtype=np.int8)
    index = {}
    packed = []
    for b in range(B):
        mb = np.asarray(mask[b, 0], dtype=np.float32)   # (q, k)
        for kt in range(SKT):
            for qc in range(SQC):
                blk = mb[qc * 512:(qc + 1) * 512, kt * 128:(kt + 1) * 128]
                if not blk.any():
                    status[b, kt, qc] = 0
                elif (blk <= -1e4).all():
                    status[b, kt, qc] = 1
                else:
                    status[b, kt, qc] = 2
                    index[(b, kt, qc)] = len(packed)
                    packed.append(np.ascontiguousarray(blk.T))  # (128k, 512q)
    if packed:
        packed_arr = np.concatenate(packed, axis=0).astype(np.float32)
    else:
        packed_arr = np.zeros((128, 512), dtype=np.float32)
    return status, index, packed_arr


def _cody_consts():
    c1 = float(np.float32(6.28125))
    r = np.float64(TWO_PI) - c1
    c2 = float(np.float32(r - np.remainder(r, 2.0 ** -24)))
    c3 = float(np.float32(np.float64(TWO_PI) - c1 - float(c2)))
    return c1, c2, c3


def _build(status, index, n_packed):
    from concourse import bacc, tile, mybir, bass_isa

    F32 = mybir.dt.float32
    F32R = mybir.dt.float32r
    BF16 = mybir.dt.bfloat16
    ACTF = mybir.ActivationFunctionType
    ALU = mybir.AluOpType
    X = mybir.AxisListType.X
    RG = [list(range(NCORES))]
    c1, c2, c3 = _cody_consts()
    ISQ = float(1.0 / np.sqrt(HD))

    nc = bacc.Bacc("TRN2", target_bir_lowering=False, debug=False,
                   num_devices=NCORES)

    xT = nc.dram_tensor("xT", [H, T], F32, kind="ExternalInput")
    wqT = nc.dram_tensor("wqT", [H, QH * HD], F32, kind="ExternalInput")
    wkT = nc.dram_tensor("wkT", [H, HD], F32, kind="ExternalInput")
    wvT = nc.dram_tensor("wvT", [H, HD], F32, kind="ExternalInput")
    woT = nc.dram_tensor("woT", [H, MSH], F32, kind="ExternalInput")
    maskP = nc.dram_tensor("maskP", [n_packed * 128, 512], F32,
                           kind="ExternalInput")
    pos = nc.dram_tensor("pos", [1, T], F32, kind="ExternalInput")
    outN = nc.dram_tensor("outN", [T, MSH], F32, kind="ExternalOutput")

    idn_c = nc.inline_tensor(np.eye(128, dtype=np.float32), name="idn_c")
    onesk_c = nc.inline_tensor(np.ones((128, 1), np.float32), name="onesk_c")
    onesm_c = nc.inline_tensor(np.ones((1, 128), np.float32), name="onesm_c")
    invf_np = (1.0 / THETA ** (np.arange(0, HD, 2, dtype=np.float32) / HD))
    invf_np = np.concatenate([invf_np, invf_np]).reshape(HD, 1)
    invf_c = nc.inline_tensor(invf_np.astype(np.float32), name="invf_c")

    NQ = float(NH * HD * H)
    NK = float(NKV * HD * H)
    NO = float(H * NH * HD)

    with tile.TileContext(nc) as tc, \
         nc.allow_low_precision(reason="float32r kernel"):
        with tc.tile_pool(name="cpool", bufs=1) as cpool, \
             tc.tile_pool(name="dbounce", bufs=1, space="DRAM") as dbounce:
            # DRAM bounce tiles (tracked by Tile for collective deps)
            arq_in = dbounce.tile([1, 4], F32, name="arq_in")
            arq_out = dbounce.tile([1, 4], F32, name="arq_out",
                                   addr_space="Shared")
            ark_in = dbounce.tile([1, 4], F32, name="ark_in")
            ark_out = dbounce.tile([1, 4], F32, name="ark_out",
                                   addr_space="Shared")
            agin = [dbounce.tile([QH * HD, 512], BF16, name=f"agin{ch}")
                    for ch in range(NTC)]
            agout = [dbounce.tile([H, 512], BF16, name=f"agout{ch}",
                                  addr_space="Shared") for ch in range(NTC)]
            wo_dram = dbounce.tile([H, MSH], BF16, name="wo_dram")

            idn = cpool.tile([128, 128], F32R, name="idn")
            nc.sync.dma_start(out=idn[:], in_=idn_c[:, :].bitcast(F32R))
            oneskr = cpool.tile([128, 1], F32R, name="oneskr")
            nc.sync.dma_start(out=oneskr[:], in_=onesk_c[:, :].bitcast(F32R))
            onesm = cpool.tile([1, 128], F32, name="onesm")
            nc.sync.dma_start(out=onesm[:], in_=onesm_c[:, :])
            invf = cpool.tile([128, 1], F32, name="invf")
            nc.sync.dma_start(out=invf[:], in_=invf_c[:, :])
            cmag = cpool.tile([128, 1], F32, name="cmag")
            nc.vector.memset(cmag[:], C_MAGIC)
            bsc = cpool.tile([128, 8], F32, name="bsc")
            grow = cpool.tile([1, 8], F32, name="grow")

            # long-lived storage: qkv outputs, raw weights (quant in place)
            qkvp = tc.alloc_tile_pool(name="qkv", bufs=1)
            qT_sb = [qkvp.tile([128, T], F32R, name=f"qT{h}")
                     for h in range(QH)]
            kT_sb = qkvp.tile([128, T], F32R, name="kT_sb")
            vT_sb = qkvp.tile([128, T], F32R, name="vT_sb")
            wbig = tc.alloc_tile_pool(name="wbig", bufs=1)
            wq_sb = wbig.tile([128, NKT * 512], F32R, name="wq_sb")
            wk_sb = wbig.tile([128, NKT * 128], F32R, name="wk_sb")
            wv_sb = wbig.tile([128, NKT * 128], F32R, name="wv_sb")

            wspecs = [(wqT, 512, wq_sb), (wkT, 128, wk_sb), (wvT, 128, wv_sb)]

            def w_f32(wi, i):
                wt, fw, dst = wspecs[wi]
                return dst[:, i * fw:(i + 1) * fw].bitcast(F32)

            with tc.tile_pool(name="tab", bufs=1) as tab:
                cos_sb = tab.tile([128, T], F32, name="cos_sb")
                ss_sb = tab.tile([128, T], F32, name="ss_sb")
                # RoPE tables first: independent of weights, runs at t=0
                # (Cody-Waite range reduction + Sin)
                with tc.tile_pool(name="rtab", bufs=2) as rtab, \
                     tc.tile_pool(name="prt", bufs=2, space="PSUM") as prt:
                    for tcn in range(NTC):
                        cs = slice(tcn * 512, (tcn + 1) * 512)
                        pchunk = rtab.tile([1, 512], F32, name=f"pos{tcn}",
                                           tag="pos")
                        nc.sync.dma_start(out=pchunk[:], in_=pos[0:1, cs])
                        pf = prt.tile([128, 512], F32, name=f"pf{tcn}",
                                      tag="pf")
                        nc.tensor.matmul(pf[:], onesm[:], pchunk[:],
                                         start=True, stop=True)
                        f_sb = rtab.tile([128, 512], F32, name=f"f{tcn}",
                                         tag="f")
                        nc.scalar.activation(f_sb[:], pf[:], ACTF.Copy,
                                             scale=invf[:])
                        k_sb = rtab.tile([128, 512], F32, name=f"kk{tcn}",
                                         tag="kk")
                        nc.vector.tensor_scalar(k_sb[:], f_sb[:],
                                                1.0 / TWO_PI, C_MAGIC,
                                                ALU.mult, ALU.add)
                        nc.vector.tensor_scalar(k_sb[:], k_sb[:], C_MAGIC,
                                                None, ALU.subtract)
                        y_sb = rtab.tile([128, 512], F32, name=f"y{tcn}",
                                         tag="y")
                        nc.vector.scalar_tensor_tensor(
                            y_sb[:], k_sb[:], -c1, f_sb[:], ALU.mult, ALU.add)
                        nc.vector.scalar_tensor_tensor(
                            y_sb[:], k_sb[:], -c2, y_sb[:], ALU.mult, ALU.add)
                        nc.vector.scalar_tensor_tensor(
                            y_sb[:], k_sb[:], -c3, y_sb[:], ALU.mult, ALU.add)
                        nc.scalar.activation(ss_sb[0:64, cs], y_sb[0:64, :],
                                             ACTF.Sin, scale=-1.0)
                        nc.scalar.activation(ss_sb[64:128, cs],
                                             y_sb[64:128, :], ACTF.Sin)
                        yc = rtab.tile([128, 512], F32, name=f"yc{tcn}",
                                       tag="yc")
                        nc.vector.tensor_scalar(yc[:], y_sb[:],
                                                float(np.pi / 2), None,
                                                ALU.add)
                        m_sb = rtab.tile([128, 512], F32, name=f"mm{tcn}",
                                         tag="mm")
                        nc.vector.tensor_scalar(m_sb[:], yc[:],
                                                float(np.pi), None, ALU.is_gt)
                        nc.vector.scalar_tensor_tensor(
                            yc[:], m_sb[:], -TWO_PI, yc[:], ALU.mult, ALU.add)
                        nc.scalar.activation(cos_sb[:, cs], yc[:], ACTF.Sin)

                with tc.tile_pool(name="gacc", bufs=1) as gacc:
                    # ---- load weights into final storage + |.| sums.
                    # DMAs alternate between the SP and ACT queues: the
                    # per-trigger cost on one sequencer otherwise paces the
                    # whole 10 MB preamble load.
                    g4 = gacc.tile([128, 4], F32, name="g4")
                    accs = [gacc.tile([128, NKT], F32, name=f"acc{wi}")
                            for wi in range(4)]
                    engs = [nc.sync, nc.scalar]
                    for i in range(NKT):
                        for wi in range(3):
                            wt, fw, dst = wspecs[wi]
                            engs[(i * 3 + wi) % 2].dma_start(
                                out=dst[:, i * fw:(i + 1) * fw],
                                in_=wt[i * 128:(i + 1) * 128, :]
                                .bitcast(F32R))
                            nc.vector.tensor_reduce(
                                accs[wi][:, i:i + 1], w_f32(wi, i), X,
                                ALU.add, apply_absolute_value=True)
                    for wi in range(3):
                        nc.vector.tensor_reduce(g4[:, wi:wi + 1], accs[wi][:],
                                                X, ALU.add)
                    gq_sb = gacc.tile([1, 4], F32, name="gq_sb")
                    nc.vector.memset(gq_sb[:], 0.0)

                    # ---- wo prepass: stream once for the abs-sum only
                    with tc.tile_pool(name="wop", bufs=3) as wop:
                        for i in range(NKT):
                            slf = wop.tile([128, 512], F32, name=f"wo_{i}",
                                           tag="wo")
                            engs[i % 2].dma_start(
                                out=slf[:],
                                in_=woT[i * 128:(i + 1) * 128, :])
                            nc.vector.tensor_reduce(
                                accs[3][:, i:i + 1], slf[:], X, ALU.add,
                                apply_absolute_value=True)
                        nc.vector.tensor_reduce(g4[:, 3:4], accs[3][:], X,
                                                ALU.add)
                        gk_sb = gacc.tile([1, 4], F32, name="gk_sb")
                        nc.vector.memset(gk_sb[:], 0.0)

                    # cross-partition totals on gpsimd (no PSUM needed),
                    # both AllReduces triggered back to back
                    gsum = gacc.tile([128, 4], F32, name="gsum")
                    nc.gpsimd.partition_all_reduce(gsum[:, 0:3], g4[:, 0:3],
                                                   128, bass_isa.ReduceOp.add)
                    nc.vector.tensor_copy(gq_sb[:, 0:3], gsum[0:1, 0:3])
                    nc.sync.dma_start(out=arq_in[:], in_=gq_sb[:])
                    nc.gpsimd.collective_compute(
                        "AllReduce", ALU.add, replica_groups=RG,
                        ins=[arq_in[:].opt()], outs=[arq_out[:].opt()])
                    nc.gpsimd.partition_all_reduce(
                        gsum[:, 3:4], g4[:, 3:4], 128, bass_isa.ReduceOp.add)
                    nc.vector.tensor_copy(gk_sb[:, 0:1], gsum[0:1, 3:4])
                    nc.sync.dma_start(out=ark_in[:], in_=gk_sb[:])
                    nc.gpsimd.collective_compute(
                        "AllReduce", ALU.add, replica_groups=RG,
                        ins=[ark_in[:].opt()], outs=[ark_out[:].opt()])
                    arq_sb = gacc.tile([1, 4], F32, name="arq_sb")
                    nc.sync.dma_start(out=arq_sb[:], in_=arq_out[:])
                    ark_sb = gacc.tile([1, 4], F32, name="ark_sb")
                    nc.sync.dma_start(out=ark_sb[:], in_=ark_out[:])

                    # gammas: grow = [1/gq, 1/gk, 1/gv, gq*gk/sqrt(d), gv,
                    #                 1/go, go]
                    gtmp = gacc.tile([1, 8], F32, name="gtmp")
                    nc.vector.tensor_scalar(gtmp[:, 0:1], arq_sb[:, 0:1],
                                            1.0 / NQ, 1e-5, ALU.mult, ALU.add)
                    nc.vector.tensor_scalar(gtmp[:, 1:3], arq_sb[:, 1:3],
                                            1.0 / NK, 1e-5, ALU.mult, ALU.add)
                    nc.vector.reciprocal(grow[:, 0:3], gtmp[:, 0:3])
                    nc.vector.tensor_mul(gtmp[:, 3:4], gtmp[:, 0:1],
                                         gtmp[:, 1:2])
                    nc.vector.tensor_scalar(grow[:, 3:4], gtmp[:, 3:4], ISQ,
                                            None, ALU.mult)
                    nc.vector.tensor_copy(grow[:, 4:5], gtmp[:, 2:3])  # gv
                    nc.gpsimd.partition_broadcast(bsc[:, 0:5], grow[:, 0:5])
                    nc.vector.tensor_scalar(gtmp[:, 4:5], ark_sb[:, 0:1],
                                            1.0 / NO, 1e-5, ALU.mult, ALU.add)
                    nc.vector.reciprocal(grow[:, 5:6], gtmp[:, 4:5])
                    nc.vector.tensor_copy(grow[:, 6:7], gtmp[:, 4:5])  # go
                    nc.gpsimd.partition_broadcast(bsc[:, 5:7], grow[:, 5:7])
                    BIQ, BIK, BIV, BSQK = (bsc[:, i:i + 1] for i in range(4))
                    BIO, BGO = bsc[:, 5:6], bsc[:, 6:7]

                    # ---- in-place ternary quant to unscaled {-1,0,+1}
                    # A on ACT, B+C on DVE; phase-1 matmuls chase this per kt.
                    with tc.tile_pool(name="qscr", bufs=2) as qscr:
                        for i in range(NKT):
                            for wi in range(3):
                                wt, fw, dst = wspecs[wi]
                                s = qscr.tile([128, fw], F32,
                                              name=f"qs_{wi}_{i}",
                                              tag=f"qs{fw}")
                                # t = w*(1/gamma) + C (exact round-half-even)
                                nc.scalar.activation(s[:], w_f32(wi, i),
                                                     ACTF.Identity,
                                                     bias=cmag[:],
                                                     scale=bsc[:, wi:wi + 1])
                                # t = min(t - C, 1)
                                nc.vector.tensor_scalar(s[:], s[:], C_MAGIC,
                                                        1.0, ALU.subtract,
                                                        ALU.min)
                                # w_t = max(t, -1)
                                nc.vector.tensor_scalar(
                                    dst[:, i * fw:(i + 1) * fw], s[:], -1.0,
                                    None, ALU.max)

                # ---- phase 1: QKV projections + RoPE; wo re-streamed and
                # ternarized to bf16 on the side (emitted late so its chain
                # never blocks the rope stream at any queue head)
                with tc.tile_pool(name="xin", bufs=4) as xin, \
                     tc.tile_pool(name="rope", bufs=2) as rope, \
                     tc.tile_pool(name="wo2", bufs=2) as wo2, \
                     tc.tile_pool(name="p1", bufs=8, space="PSUM") as p1:

                    def wo_quant(i):
                        src = wo2.tile([128, 512], F32, name=f"wr_{i}",
                                       tag="wr")
                        nc.scalar.dma_start(
                            out=src[:], in_=woT[i * 128:(i + 1) * 128, :])
                        s = wo2.tile([128, 512], F32, name=f"ws_{i}",
                                     tag="ws")
                        nc.scalar.activation(s[:], src[:], ACTF.Identity,
                                             bias=cmag[:], scale=BIO)
                        nc.vector.tensor_scalar(s[:], s[:], C_MAGIC, 1.0,
                                                ALU.subtract, ALU.min)
                        wt = wo2.tile([128, 512], BF16, name=f"wt_{i}",
                                      tag="wt")
                        nc.vector.tensor_scalar(wt[:], s[:], -1.0, None,
                                                ALU.max)
                        nc.scalar.dma_start(
                            out=wo_dram[i * 128:(i + 1) * 128, :], in_=wt[:])

                    for tcn in range(NTC):
                        cs = slice(tcn * 512, (tcn + 1) * 512)
                        pq = [p1.tile([128, 512], F32, name=f"pq{tcn}_{h}",
                                      tag="p1") for h in range(QH)]
                        pk = p1.tile([128, 512], F32, name=f"pk{tcn}",
                                     tag="p1")
                        pv = p1.tile([128, 512], F32, name=f"pv{tcn}",
                                     tag="p1")
                        for kt in range(NKT):
                            xt = xin.tile([128, 512], F32R,
                                          name=f"x{tcn}_{kt}", tag="xt")
                            nc.sync.dma_start(
                                out=xt[:],
                                in_=xT[kt * 128:(kt + 1) * 128, cs]
                                .bitcast(F32R))
                            st, sp = (kt == 0), (kt == NKT - 1)
                            for h in range(QH):
                                nc.tensor.matmul(
                                    pq[h][:],
                                    wq_sb[:, kt * 512 + h * 128:
                                          kt * 512 + (h + 1) * 128],
                                    xt[:], start=st, stop=sp,
                                    skip_group_check=True)
                            nc.tensor.matmul(
                                pk[:], wk_sb[:, kt * 128:(kt + 1) * 128],
                                xt[:], start=st, stop=sp,
                                skip_group_check=True)
                            nc.tensor.matmul(
                                pv[:], wv_sb[:, kt * 128:(kt + 1) * 128],
                                xt[:], start=st, stop=sp,
                                skip_group_check=True)

                        def rope_apply(psrc, dst_ap, tg):
                            m1 = rope.tile([128, 512], F32, name=f"m1{tg}",
                                           tag="m1")
                            nc.vector.tensor_mul(m1[:], psrc[:], cos_sb[:, cs])
                            m2 = rope.tile([128, 512], F32, name=f"m2{tg}",
                                           tag="m2")
                            nc.vector.tensor_mul(m2[0:64, :], psrc[64:128, :],
                                                 ss_sb[0:64, cs])
                            nc.vector.tensor_mul(m2[64:128, :], psrc[0:64, :],
                                                 ss_sb[64:128, cs])
                            nc.vector.tensor_add(dst_ap, m1[:], m2[:])
                        for h in range(QH):
                            rope_apply(pq[h], qT_sb[h][:, cs], f"_{tcn}_{h}")
                        rope_apply(pk, kT_sb[:, cs], f"k_{tcn}")
                        nc.scalar.copy(vT_sb[:, cs], pv[:])
                        if tcn >= 2:
                            for i in range((tcn - 2) * 16, (tcn - 1) * 16):
                                wo_quant(i)

            wbig.release()  # free wq/wk/wv SBUF before attention
            # prefetch quantized bf16 wo back into SBUF during attention
            w3 = tc.alloc_tile_pool(name="w3", bufs=1)
            wo_sb = w3.tile([128, NKT * MSH], BF16, name="wo_sb")
            for i in range(NKT):
                nc.sync.dma_start(out=wo_sb[:, i * MSH:(i + 1) * MSH],
                                  in_=wo_dram[i * 128:(i + 1) * 128, :])

            # ---- phase 2: attention (chunked bf16 AllGather per (b,qc)) ----
            with tc.tile_pool(name="vnatp", bufs=2) as vnatp, \
                 tc.tile_pool(name="epool", bufs=8) as epool, \
                 tc.tile_pool(name="mpool", bufs=1) as mpool, \
                 tc.tile_pool(name="aop", bufs=4) as aop, \
                 tc.tile_pool(name="zpool", bufs=2) as zpool, \
                 tc.tile_pool(name="ps_s", bufs=2, space="PSUM") as ps_s, \
                 tc.tile_pool(name="ps_o", bufs=2, space="PSUM") as ps_o, \
                 tc.tile_pool(name="ps_x", bufs=2, space="PSUM") as ps_x:
                for b in range(B):
                    boff = b * S
                    vnat = vnatp.tile([128, S], F32R, name=f"vnat{b}",
                                      tag="vnat")
                    for kt in range(SKT):
                        ptr = ps_x.tile([128, 128], F32R, name=f"ptr{b}_{kt}",
                                        tag="ptr")
                        nc.tensor.transpose(
                            ptr[:],
                            vT_sb[:, boff + kt * 128:boff + (kt + 1) * 128],
                            idn[:])
                        nc.scalar.copy(vnat[:, kt * 128:(kt + 1) * 128],
                                       ptr[:].bitcast(F32))
                    for qc in range(SQC):
                        ch = b * SQC + qc
                        # mask tiles shared across the 4 heads
                        mtiles = {}
                        for kt in range(SKT):
                            if status[b, kt, qc] == 2:
                                mi = index[(b, kt, qc)]
                                mt_ = mpool.tile([128, 512], F32,
                                                 name=f"mt{b}{qc}{kt}",
                                                 tag=f"mt{kt}")
                                nc.sync.dma_start(
                                    out=mt_[:],
                                    in_=maskP[mi * 128:(mi + 1) * 128, :])
                                mtiles[kt] = mt_
                        kts = [kt for kt in range(SKT)
                               if status[b, kt, qc] != 1]
                        assert kts, "fully-masked softmax row"
                        for h in range(QH):
                            qsl = qT_sb[h][:, boff + qc * 512:
                                           boff + (qc + 1) * 512]
                            pz = ps_x.tile([1, 512], F32, name=f"pz{b}{h}{qc}",
                                           tag="pz")
                            po = ps_o.tile([128, 512], F32,
                                           name=f"po{b}{h}{qc}", tag="po")
                            for i, kt in enumerate(kts):
                                ps_ = ps_s.tile([128, 512], F32,
                                                name=f"s{b}{h}{qc}{kt}",
                                                tag="ps")
                                nc.tensor.matmul(
                                    ps_[:],
                                    kT_sb[:, boff + kt * 128:
                                          boff + (kt + 1) * 128],
                                    qsl, start=True, stop=True,
                                    skip_group_check=True)
                                e = epool.tile([128, 512], F32R,
                                               name=f"e{b}{h}{qc}{kt}",
                                               tag="e")
                                if kt in mtiles:
                                    # scores*sqk + mask, then exp
                                    nc.vector.scalar_tensor_tensor(
                                        ps_[:], ps_[:], BSQK, mtiles[kt][:],
                                        ALU.mult, ALU.add)
                                    nc.scalar.activation(e[:], ps_[:],
                                                         ACTF.Exp)
                                else:
                                    nc.scalar.activation(e[:], ps_[:],
                                                         ACTF.Exp, scale=BSQK)
                                fst = (i == 0)
                                lst = (i == len(kts) - 1)
                                nc.tensor.matmul(
                                    pz[:], oneskr[:], e[:], start=fst,
                                    stop=lst, skip_group_check=True)
                                nc.tensor.matmul(
                                    po[:],
                                    vnat[:, kt * 128:(kt + 1) * 128], e[:],
                                    start=fst, stop=lst,
                                    skip_group_check=True)
                            # zr = gamma_v / z, broadcast to 128 partitions
                            zr = zpool.tile([1, 512], F32,
                                            name=f"zr{b}{h}{qc}", tag="zr")
                            nc.vector.reciprocal(zr[:], pz[:])
                            nc.vector.tensor_scalar(zr[:], zr[:],
                                                    grow[0:1, 4:5], None,
                                                    ALU.mult)
                            zb = zpool.tile([128, 512], F32,
                                            name=f"zb{b}{h}{qc}", tag="zb")
                            nc.gpsimd.partition_broadcast(zb[:], zr[:])
                            ao = aop.tile([128, 512], BF16,
                                          name=f"ao{b}{h}{qc}", tag="ao")
                            nc.vector.tensor_mul(ao[:], po[:], zb[:])
                            nc.sync.dma_start(
                                out=agin[ch][h * 128:(h + 1) * 128, :],
                                in_=ao[:])
                        nc.gpsimd.collective_compute(
                            "AllGather", ALU.bypass, replica_groups=RG,
                            ins=[agin[ch][:].opt()],
                            outs=[agout[ch][:].opt()])

            # ---- phase 3: o_proj in bf16 (out in natural [token, m]) ----
            with tc.tile_pool(name="a3", bufs=1) as a3, \
                 tc.tile_pool(name="o3", bufs=2) as o3, \
                 tc.tile_pool(name="p3", bufs=2, space="PSUM") as p3:
                for ch in range(NTC):
                    ats = []
                    for kt in range(NKT):
                        at = a3.tile([128, 512], BF16, name=f"at{ch}_{kt}",
                                     tag=f"at{kt}")
                        nc.sync.dma_start(
                            out=at[:],
                            in_=agout[ch][kt * 128:(kt + 1) * 128, :])
                        ats.append(at)
                    for tt in range(4):
                        pout = p3.tile([128, 512], F32, name=f"po3_{ch}{tt}",
                                       tag="pout")
                        for kt in range(NKT):
                            nc.tensor.matmul(
                                pout[:],
                                ats[kt][:, tt * 128:(tt + 1) * 128],
                                wo_sb[:, kt * MSH:(kt + 1) * MSH],
                                start=(kt == 0), stop=(kt == NKT - 1),
                                skip_group_check=True)
                        osb = o3.tile([128, 512], F32, name=f"osb{ch}{tt}",
                                      tag="osb")
                        nc.scalar.activation(osb[:], pout[:], ACTF.Copy,
                                             scale=BGO)
                        nc.sync.dma_start(
                            out=outN[ch * 512 + tt * 128:
                                     ch * 512 + (tt + 1) * 128, :],
                            in_=osb[:])
            w3.release()
            qkvp.release()

    nc.compile()
    return nc


def kernel(hidden_states, Wq, Wk, Wv, Wo, attention_mask, position_ids):
    from concourse.bass_utils import run_bass_kernel_spmd
    from concourse.bass_interp import get_hw_module

    hs = np.ascontiguousarray(np.asarray(hidden_states, dtype=np.float32))
    Wq = np.asarray(Wq, dtype=np.float32)
    Wk = np.asarray(Wk, dtype=np.float32)
    Wv = np.asarray(Wv, dtype=np.float32)
    Wo = np.asarray(Wo, dtype=np.float32)
    mask = np.asarray(attention_mask, dtype=np.float32)
    posf = np.ascontiguousarray(
        np.asarray(position_ids).reshape(1, T).astype(np.float32))

    status, index, packed = _classify_mask(mask)
    n_packed = packed.shape[0] // 128

    key = (status.tobytes(), n_packed)
    if key not in _cache:
        nc = _build(status, index, n_packed)
        nc.m = get_hw_module(nc.m)
        _cache[key] = nc
    nc = _cache[key]

    xT = np.ascontiguousarray(hs.reshape(T, H).T)
    in_maps = []
    for c in range(NCORES):
        in_maps.append({
            "xT": xT,
            "wqT": np.ascontiguousarray(
                Wq[c * QH * HD:(c + 1) * QH * HD, :].T),
            "wkT": np.ascontiguousarray(Wk[c * HD:(c + 1) * HD, :].T),
            "wvT": np.ascontiguousarray(Wv[c * HD:(c + 1) * HD, :].T),
            "woT": np.ascontiguousarray(Wo[c * MSH:(c + 1) * MSH, :].T),
            "maskP": packed,
            "pos": posf,
        })
    res = run_bass_kernel_spmd(nc, in_maps, core_ids=list(range(NCORES)),
                               trace=bool(os.environ.get("BITNET_TRACE")))
    global last_exec_time_ns
    last_exec_time_ns = res.exec_time_ns
    out = np.concatenate(
        [res.results[c]["outN"] for c in range(NCORES)], axis=1)  # (T, H)
    return np.ascontiguousarray(out).reshape(B, S, H).astype(np.float32)


# revision 21
# speedup vs baseline: 1.2636x; 1.0540x over previous
"""BitNet attention block on 8 TRN2 NeuronCores (tensor-parallel over heads).

Self-contained: kernel(**inputs) takes full inputs, shards internally,
runs one SPMD Bass program on cores 0-7, reassembles the full output.

Sharding: core c owns Q heads [4c,4c+4), KV head c, o_proj output dims
[512c, 512c+512). Attention is fully local per core. Cross-core comms:
two tiny AllReduces for the BitNet absmean gammas and four chunked
bf16 AllGathers (one per (batch, 512-token q-chunk)) of the attention
output so each core can compute its o_proj output-dim shard.

Key structure (vs a naive phase-serial version):
- Weights arrive host-pre-tiled as [128, NKT*fw] so the preamble load
  is a handful of big contiguous DMAs straight into SBUF matmul
  storage, abs-reduced from there and ternarized IN PLACE to unscaled
  {-1,0,+1} right after the gamma AllReduce. Weights live in 4-kt
  group tiles so the projection matmuls chase the quantization at
  group granularity (Tile deps are per-tile).
- All gamma scale factors are folded downstream instead of into the
  weights: gamma_q*gamma_k/sqrt(HD) into the softmax exp scale,
  gamma_v into the 1/z correction, gamma_o into the o_proj PSUM copy.
  The ternary weights are exact in every dtype, so o_proj runs bf16.
- Wo is abs-reduced in the preamble, then re-streamed + ternarized to
  bf16 during phase 1 (engines idle then), staged through DRAM, and
  prefetched into SBUF before o_proj needs it.
- Attention is software-pipelined: score matmuls run 3 k-tiles ahead
  of the pz/po accumulation matmuls so the PE never waits on the
  mask-add/exp chain. exp output and V^T are bf16 (post-softmax path,
  exact-enough), enabling fast weight load on the pz/po matmuls.
- GpSimd does only partition broadcasts/reductions, the gamma bounce
  DMAs and collective triggers (its elementwise path is ~8-15x slower
  than DVE). Engine FIFOs are in-order, so emission order keeps every
  queue head runnable (rope tables first, gammas before quant, wo
  chains late in phase 1).
"""
import os
import sys
sys.path.insert(0, "/opt/trn_rl_repo")
import numpy as np

B, S, H = 2, 1024, 4096
NH, NKV, HD = 32, 8, 128
NCORES = 8
T = B * S
QH = NH // NCORES          # 4 q-heads per core
MSH = H // NCORES          # 512 o_proj out-dims per core
THETA = 10000.0
C_MAGIC = 12582912.0       # 1.5 * 2**23: (x + C) - C == round-half-even(x)
TWO_PI = 6.283185307179586
NKT = H // 128             # 32 contraction tiles
NGRP = 8                   # weight quant/load groups
GKT = NKT // NGRP          # 4 kt per group
NTC = T // 512             # 4 token chunks
SKT = S // 128             # 8 score k-tiles per batch
SQC = S // 512             # 2 q-chunks per batch
LOOK = 3                   # attention score lookahead (software pipeline)

_cache = {}
last_exec_time_ns = None


def _classify_mask(mask):
    """Per (b, kt, qc) [128k x 512q] block: 0 zero, 1 masked-out, 2 general.

    A block whose mask values are all <= -1e4 contributes exactly 0 to the
    fp32 softmax (exp underflows to 0.0), so it is skipped entirely.
    """
    status = np.empty((B, SKT, SQC), dtype=np.int8)
    index = {}
    packed = []
    for b in range(B):
        mb = np.asarray(mask[b, 0], dtype=np.float32)   # (q, k)
        for kt in range(SKT):
            for qc in range(SQC):
                blk = mb[qc * 512:(qc + 1) * 512, kt * 128:(kt + 1) * 128]
                if not blk.any():
                    status[b, kt, qc] = 0
                elif (blk <= -1e4).all():
                    status[b, kt, qc] = 1
                else:
                    status[b, kt, qc] = 2
                    index[(b, kt, qc)] = len(packed)
                    packed.append(np.ascontiguousarray(blk.T))  # (128k, 512q)
    if packed:
        packed_arr = np.concatenate(packed, axis=0).astype(np.float32)
    else:
        packed_arr = np.zeros((128, 512), dtype=np.float32)
    return status, index, packed_arr


def _cody_consts():
    c1 = float(np.float32(6.28125))
    r = np.float64(TWO_PI) - c1
    c2 = float(np.float32(r - np.remainder(r, 2.0 ** -24)))
    c3 = float(np.float32(np.float64(TWO_PI) - c1 - float(c2)))
    return c1, c2, c3


def _build(status, index, n_packed):
    from concourse import bacc, tile, mybir, bass_isa

    F32 = mybir.dt.float32
    F32R = mybir.dt.float32r
    BF16 = mybir.dt.bfloat16
    ACTF = mybir.ActivationFunctionType
    ALU = mybir.AluOpType
    X = mybir.AxisListType.X
    RG = [list(range(NCORES))]
    c1, c2, c3 = _cody_consts()
    ISQ = float(1.0 / np.sqrt(HD))

    nc = bacc.Bacc("TRN2", target_bir_lowering=False, debug=False,
                   num_devices=NCORES)

    # weights are host-pre-tiled: wXt[p, kt*fw + c] = W_shard.T[kt*128+p, c]
    xT = nc.dram_tensor("xT", [H, T], F32, kind="ExternalInput")
    wqt = nc.dram_tensor("wqt", [128, NKT * 512], F32, kind="ExternalInput")
    wkt = nc.dram_tensor("wkt", [128, NKT * 128], F32, kind="ExternalInput")
    wvt = nc.dram_tensor("wvt", [128, NKT * 128], F32, kind="ExternalInput")
    wot = nc.dram_tensor("wot", [128, NKT * 512], F32, kind="ExternalInput")
    maskP = nc.dram_tensor("maskP", [n_packed * 128, 512], F32,
                           kind="ExternalInput")
    pos = nc.dram_tensor("pos", [1, T], F32, kind="ExternalInput")
    outN = nc.dram_tensor("outN", [T, MSH], F32, kind="ExternalOutput")

    idn_c = nc.inline_tensor(np.eye(128, dtype=np.float32), name="idn_c")
    onesm_c = nc.inline_tensor(np.ones((1, 128), np.float32), name="onesm_c")
    invf_np = (1.0 / THETA ** (np.arange(0, HD, 2, dtype=np.float32) / HD))
    invf_np = np.concatenate([invf_np, invf_np]).reshape(HD, 1)
    invf_c = nc.inline_tensor(invf_np.astype(np.float32), name="invf_c")

    NQ = float(NH * HD * H)
    NK = float(NKV * HD * H)
    NO = float(H * NH * HD)
    GW = [512 * GKT, 128 * GKT, 128 * GKT]   # group widths q,k,v

    with tile.TileContext(nc) as tc, \
         nc.allow_low_precision(reason="float32r kernel"):
        with tc.tile_pool(name="cpool", bufs=1) as cpool, \
             tc.tile_pool(name="dbounce", bufs=1, space="DRAM") as dbounce:
            # DRAM bounce tiles (tracked by Tile for collective deps)
            arq_in = dbounce.tile([1, 4], F32, name="arq_in")
            arq_out = dbounce.tile([1, 4], F32, name="arq_out",
                                   addr_space="Shared")
            ark_in = dbounce.tile([1, 4], F32, name="ark_in")
            ark_out = dbounce.tile([1, 4], F32, name="ark_out",
                                   addr_space="Shared")
            agin = [dbounce.tile([QH * HD, 512], BF16, name=f"agin{ch}")
                    for ch in range(NTC)]
            agout = [dbounce.tile([H, 512], BF16, name=f"agout{ch}",
                                  addr_space="Shared") for ch in range(NTC)]
            wo_dram = dbounce.tile([128, NKT * 512], BF16, name="wo_dram")

            idn = cpool.tile([128, 128], F32R, name="idn")
            nc.sync.dma_start(out=idn[:], in_=idn_c[:, :].bitcast(F32R))
            onesb = cpool.tile([128, 1], BF16, name="onesb")
            nc.vector.memset(onesb[:], 1.0)
            onesm = cpool.tile([1, 128], F32, name="onesm")
            nc.sync.dma_start(out=onesm[:], in_=onesm_c[:, :])
            invf = cpool.tile([128, 1], F32, name="invf")
            nc.sync.dma_start(out=invf[:], in_=invf_c[:, :])
            cmag = cpool.tile([128, 1], F32, name="cmag")
            nc.vector.memset(cmag[:], C_MAGIC)
            ncmag = cpool.tile([128, 1], F32, name="ncmag")
            nc.vector.memset(ncmag[:], -C_MAGIC)
            bsc = cpool.tile([128, 8], F32, name="bsc")
            grow = cpool.tile([1, 8], F32, name="grow")

            # long-lived storage: qkv outputs, weight group tiles
            qkvp = tc.alloc_tile_pool(name="qkv", bufs=1)
            qT_sb = [qkvp.tile([128, T], F32R, name=f"qT{h}")
                     for h in range(QH)]
            kT_sb = qkvp.tile([128, T], F32R, name="kT_sb")
            vT_sb = qkvp.tile([128, T], F32R, name="vT_sb")
            wbig = tc.alloc_tile_pool(name="wbig", bufs=1)
            wgrp = [[wbig.tile([128, GW[wi]], F32R, name=f"w{wi}_{g}",
                               tag=f"w{wi}_{g}") for g in range(NGRP)]
                    for wi in range(3)]

            def w_f32(wi, g):
                return wgrp[wi][g][:, :].bitcast(F32)

            def w_kt(wi, kt):
                fw = GW[wi] // GKT
                g, j = kt // GKT, kt % GKT
                return wgrp[wi][g][:, j * fw:(j + 1) * fw]

            wsrcs = [wqt, wkt, wvt]

            with tc.tile_pool(name="tab", bufs=1) as tab:
                cos_sb = tab.tile([128, T], F32, name="cos_sb")
                ss_sb = tab.tile([128, T], F32, name="ss_sb")
                # RoPE tables first: independent of weights, runs at t=0
                # (Cody-Waite range reduction + Sin)
                with tc.tile_pool(name="rtab", bufs=2) as rtab, \
                     tc.tile_pool(name="prt", bufs=2, space="PSUM") as prt:
                    for tcn in range(NTC):
                        cs = slice(tcn * 512, (tcn + 1) * 512)
                        pchunk = rtab.tile([1, 512], F32, name=f"pos{tcn}",
                                           tag="pos")
                        nc.sync.dma_start(out=pchunk[:], in_=pos[0:1, cs])
                        pf = prt.tile([128, 512], F32, name=f"pf{tcn}",
                                      tag="pf")
                        nc.tensor.matmul(pf[:], onesm[:], pchunk[:],
                                         start=True, stop=True)
                        f_sb = rtab.tile([128, 512], F32, name=f"f{tcn}",
                                         tag="f")
                        nc.scalar.activation(f_sb[:], pf[:], ACTF.Copy,
                                             scale=invf[:])
                        k_sb = rtab.tile([128, 512], F32, name=f"kk{tcn}",
                                         tag="kk")
                        nc.vector.tensor_scalar(k_sb[:], f_sb[:],
                                                1.0 / TWO_PI, C_MAGIC,
                                                ALU.mult, ALU.add)
                        nc.vector.tensor_scalar(k_sb[:], k_sb[:], C_MAGIC,
                                                None, ALU.subtract)
                        y_sb = rtab.tile([128, 512], F32, name=f"y{tcn}",
                                         tag="y")
                        nc.vector.scalar_tensor_tensor(
                            y_sb[:], k_sb[:], -c1, f_sb[:], ALU.mult, ALU.add)
                        nc.vector.scalar_tensor_tensor(
                            y_sb[:], k_sb[:], -c2, y_sb[:], ALU.mult, ALU.add)
                        nc.vector.scalar_tensor_tensor(
                            y_sb[:], k_sb[:], -c3, y_sb[:], ALU.mult, ALU.add)
                        nc.scalar.activation(ss_sb[0:64, cs], y_sb[0:64, :],
                                             ACTF.Sin, scale=-1.0)
                        nc.scalar.activation(ss_sb[64:128, cs],
                                             y_sb[64:128, :], ACTF.Sin)
                        yc = rtab.tile([128, 512], F32, name=f"yc{tcn}",
                                       tag="yc")
                        nc.vector.tensor_scalar(yc[:], y_sb[:],
                                                float(np.pi / 2), None,
                                                ALU.add)
                        m_sb = rtab.tile([128, 512], F32, name=f"mm{tcn}",
                                         tag="mm")
                        nc.vector.tensor_scalar(m_sb[:], yc[:],
                                                float(np.pi), None, ALU.is_gt)
                        nc.vector.scalar_tensor_tensor(
                            yc[:], m_sb[:], -TWO_PI, yc[:], ALU.mult, ALU.add)
                        nc.scalar.activation(cos_sb[:, cs], yc[:], ACTF.Sin)

                with tc.tile_pool(name="gacc", bufs=1) as gacc:
                    # ---- load weights into final storage + |.| sums,
                    # one big DMA per 4-kt group, alternating SP/ACT queues
                    g4 = gacc.tile([128, 4], F32, name="g4")
                    accs = [gacc.tile([128, NGRP], F32, name=f"acc{wi}")
                            for wi in range(4)]
                    engs = [nc.sync, nc.scalar]
                    for g in range(NGRP):
                        for wi in range(3):
                            engs[(g * 3 + wi) % 2].dma_start(
                                out=wgrp[wi][g][:, :],
                                in_=wsrcs[wi][:, g * GW[wi]:(g + 1) * GW[wi]]
                                .bitcast(F32R))
                            nc.vector.tensor_reduce(
                                accs[wi][:, g:g + 1], w_f32(wi, g), X,
                                ALU.add, apply_absolute_value=True)
                    for wi in range(3):
                        nc.vector.tensor_reduce(g4[:, wi:wi + 1], accs[wi][:],
                                                X, ALU.add)
                    gq_sb = gacc.tile([1, 4], F32, name="gq_sb")
                    nc.vector.memset(gq_sb[:], 0.0)

                    # ---- wo prepass: stream once for the abs-sum only
                    with tc.tile_pool(name="wop", bufs=2) as wop:
                        for g in range(NGRP):
                            slf = wop.tile([128, 512 * GKT], F32,
                                           name=f"wo_{g}", tag="wo")
                            engs[g % 2].dma_start(
                                out=slf[:],
                                in_=wot[:, g * 512 * GKT:(g + 1) * 512 * GKT])
                            nc.vector.tensor_reduce(
                                accs[3][:, g:g + 1], slf[:], X, ALU.add,
                                apply_absolute_value=True)
                        nc.vector.tensor_reduce(g4[:, 3:4], accs[3][:], X,
                                                ALU.add)
                        gk_sb = gacc.tile([1, 4], F32, name="gk_sb")
                        nc.vector.memset(gk_sb[:], 0.0)

                    # cross-partition totals + gamma AllReduces; the bounce
                    # DMAs ride the otherwise-empty gpsimd queue
                    gsum = gacc.tile([128, 4], F32, name="gsum")
                    nc.gpsimd.partition_all_reduce(gsum[:, 0:3], g4[:, 0:3],
                                                   128, bass_isa.ReduceOp.add)
                    nc.vector.tensor_copy(gq_sb[:, 0:3], gsum[0:1, 0:3])
                    nc.gpsimd.dma_start(out=arq_in[:], in_=gq_sb[:])
                    nc.gpsimd.collective_compute(
                        "AllReduce", ALU.add, replica_groups=RG,
                        ins=[arq_in[:].opt()], outs=[arq_out[:].opt()])
                    nc.gpsimd.partition_all_reduce(
                        gsum[:, 3:4], g4[:, 3:4], 128, bass_isa.ReduceOp.add)
                    nc.vector.tensor_copy(gk_sb[:, 0:1], gsum[0:1, 3:4])
                    nc.gpsimd.dma_start(out=ark_in[:], in_=gk_sb[:])
                    nc.gpsimd.collective_compute(
                        "AllReduce", ALU.add, replica_groups=RG,
                        ins=[ark_in[:].opt()], outs=[ark_out[:].opt()])
                    arq_sb = gacc.tile([1, 4], F32, name="arq_sb")
                    nc.gpsimd.dma_start(out=arq_sb[:], in_=arq_out[:])
                    ark_sb = gacc.tile([1, 4], F32, name="ark_sb")
                    nc.gpsimd.dma_start(out=ark_sb[:], in_=ark_out[:])

                    # gammas: grow = [1/gq, 1/gk, 1/gv, gq*gk/sqrt(d), gv,
                    #                 1/go, go]
                    gtmp = gacc.tile([1, 8], F32, name="gtmp")
                    nc.vector.tensor_scalar(gtmp[:, 0:1], arq_sb[:, 0:1],
                                            1.0 / NQ, 1e-5, ALU.mult, ALU.add)
                    nc.vector.tensor_scalar(gtmp[:, 1:3], arq_sb[:, 1:3],
                                            1.0 / NK, 1e-5, ALU.mult, ALU.add)
                    nc.vector.reciprocal(grow[:, 0:3], gtmp[:, 0:3])
                    nc.vector.tensor_mul(gtmp[:, 3:4], gtmp[:, 0:1],
                                         gtmp[:, 1:2])
                    nc.vector.tensor_scalar(grow[:, 3:4], gtmp[:, 3:4], ISQ,
                                            None, ALU.mult)
                    nc.vector.tensor_copy(grow[:, 4:5], gtmp[:, 2:3])  # gv
                    nc.gpsimd.partition_broadcast(bsc[:, 0:5], grow[:, 0:5])
                    nc.vector.tensor_scalar(gtmp[:, 4:5], ark_sb[:, 0:1],
                                            1.0 / NO, 1e-5, ALU.mult, ALU.add)
                    nc.vector.reciprocal(grow[:, 5:6], gtmp[:, 4:5])
                    nc.vector.tensor_copy(grow[:, 6:7], gtmp[:, 4:5])  # go
                    nc.gpsimd.partition_broadcast(bsc[:, 5:7], grow[:, 5:7])
                    BSQK = bsc[:, 3:4]
                    BIO, BGO = bsc[:, 5:6], bsc[:, 6:7]

                    # ---- in-place ternary quant to unscaled {-1,0,+1},
                    # per 4-kt group; phase-1 matmuls chase this per group.
                    # A(q) on ACT + B/C on DVE; k/v A on DVE, B on ACT.
                    with tc.tile_pool(name="qscr", bufs=2) as qscr:
                        for g in range(NGRP):
                            for wi in range(3):
                                s = qscr.tile([128, GW[wi]], F32,
                                              name=f"qs_{wi}_{g}",
                                              tag=f"qs{GW[wi]}")
                                inv_ap = bsc[:, wi:wi + 1]
                                # t = w*(1/gamma) + C (exact round-half-even)
                                if wi == 0:
                                    nc.scalar.activation(s[:], w_f32(wi, g),
                                                         ACTF.Identity,
                                                         bias=cmag[:],
                                                         scale=inv_ap)
                                    nc.vector.tensor_scalar(
                                        s[:], s[:], C_MAGIC, 1.0,
                                        ALU.subtract, ALU.min)
                                else:
                                    nc.vector.tensor_scalar(
                                        s[:], w_f32(wi, g), inv_ap, C_MAGIC,
                                        ALU.mult, ALU.add)
                                    nc.scalar.activation(s[:], s[:],
                                                         ACTF.Identity,
                                                         bias=ncmag[:])
                                    nc.vector.tensor_scalar(
                                        s[:], s[:], 1.0, None, ALU.min)
                                # w_t = clip to [-1, 1]
                                nc.vector.tensor_scalar(
                                    wgrp[wi][g][:, :], s[:], -1.0, None,
                                    ALU.max)

                # ---- phase 1: QKV projections + RoPE; wo re-streamed and
                # ternarized to bf16 on the side (emitted late so its chain
                # never blocks the rope stream at any queue head)
                with tc.tile_pool(name="xin", bufs=4) as xin, \
                     tc.tile_pool(name="rope", bufs=2) as rope, \
                     tc.tile_pool(name="wo2", bufs=2) as wo2, \
                     tc.tile_pool(name="p1", bufs=8, space="PSUM") as p1:

                    def wo_quant(g):
                        w0, w1 = g * 1024, (g + 1) * 1024
                        src = wo2.tile([128, 1024], F32, name=f"wr_{g}",
                                       tag="wr")
                        nc.scalar.dma_start(out=src[:], in_=wot[:, w0:w1])
                        s = wo2.tile([128, 1024], F32, name=f"ws_{g}",
                                     tag="ws")
                        nc.scalar.activation(s[:], src[:], ACTF.Identity,
                                             bias=cmag[:], scale=BIO)
                        nc.vector.tensor_scalar(s[:], s[:], C_MAGIC, 1.0,
                                                ALU.subtract, ALU.min)
                        wt = wo2.tile([128, 1024], BF16, name=f"wt_{g}",
                                      tag="wt")
                        nc.vector.tensor_scalar(wt[:], s[:], -1.0, None,
                                                ALU.max)
                        nc.scalar.dma_start(out=wo_dram[:, w0:w1], in_=wt[:])

                    for tcn in range(NTC):
                        cs = slice(tcn * 512, (tcn + 1) * 512)
                        pq = [p1.tile([128, 512], F32, name=f"pq{tcn}_{h}",
                                      tag="p1") for h in range(QH)]
                        pk = p1.tile([128, 512], F32, name=f"pk{tcn}",
                                     tag="p1")
                        pv = p1.tile([128, 512], F32, name=f"pv{tcn}",
                                     tag="p1")
                        for kt in range(NKT):
                            xt = xin.tile([128, 512], F32R,
                                          name=f"x{tcn}_{kt}", tag="xt")
                            nc.sync.dma_start(
                                out=xt[:],
                                in_=xT[kt * 128:(kt + 1) * 128, cs]
                                .bitcast(F32R))
                            st, sp = (kt == 0), (kt == NKT - 1)
                            wq_kt = w_kt(0, kt)
                            for h in range(QH):
                                nc.tensor.matmul(
                                    pq[h][:],
                                    wq_kt[:, h * 128:(h + 1) * 128],
                                    xt[:], start=st, stop=sp,
                                    skip_group_check=True)
                            nc.tensor.matmul(
                                pk[:], w_kt(1, kt), xt[:], start=st, stop=sp,
                                skip_group_check=True)
                            nc.tensor.matmul(
                                pv[:], w_kt(2, kt), xt[:], start=st, stop=sp,
                                skip_group_check=True)

                        def rope_apply(psrc, dst_ap, tg):
                            m1 = rope.tile([128, 512], F32, name=f"m1{tg}",
                                           tag="m1")
                            nc.vector.tensor_mul(m1[:], psrc[:], cos_sb[:, cs])
                            m2 = rope.tile([128, 512], F32, name=f"m2{tg}",
                                           tag="m2")
                            nc.vector.tensor_mul(m2[0:64, :], psrc[64:128, :],
                                                 ss_sb[0:64, cs])
                            nc.vector.tensor_mul(m2[64:128, :], psrc[0:64, :],
                                                 ss_sb[64:128, cs])
                            nc.vector.tensor_add(dst_ap, m1[:], m2[:])
                        for h in range(QH):
                            rope_apply(pq[h], qT_sb[h][:, cs], f"_{tcn}_{h}")
                        rope_apply(pk, kT_sb[:, cs], f"k_{tcn}")
                        nc.scalar.copy(vT_sb[:, cs], pv[:])
                        if tcn >= 2:
                            for g in range((tcn - 2) * 8, (tcn - 1) * 8):
                                wo_quant(g)

            wbig.release()  # free weight SBUF before attention
            # prefetch quantized bf16 wo back into SBUF during attention
            w3 = tc.alloc_tile_pool(name="w3", bufs=1)
            wo_sb = w3.tile([128, NKT * MSH], BF16, name="wo_sb")
            for g in range(NGRP):
                w0, w1 = g * 512 * GKT, (g + 1) * 512 * GKT
                nc.sync.dma_start(out=wo_sb[:, w0:w1], in_=wo_dram[:, w0:w1])

            # ---- phase 2: attention (chunked bf16 AllGather per (b,qc)) ----
            with tc.tile_pool(name="vnatp", bufs=2) as vnatp, \
                 tc.tile_pool(name="epool", bufs=8) as epool, \
                 tc.tile_pool(name="mpool", bufs=1) as mpool, \
                 tc.tile_pool(name="aop", bufs=4) as aop, \
                 tc.tile_pool(name="zpool", bufs=2) as zpool, \
                 tc.tile_pool(name="ps_s", bufs=4, space="PSUM") as ps_s, \
                 tc.tile_pool(name="ps_o", bufs=2, space="PSUM") as ps_o, \
                 tc.tile_pool(name="ps_x", bufs=1, space="PSUM") as ps_x:
                for b in range(B):
                    boff = b * S
                    vnat = vnatp.tile([128, S], BF16, name=f"vnat{b}",
                                      tag="vnat")
                    for kt in range(SKT):
                        ptr = ps_x.tile([128, 128], F32R, name=f"ptr{b}_{kt}",
                                        tag="ptr")
                        nc.tensor.transpose(
                            ptr[:],
                            vT_sb[:, boff + kt * 128:boff + (kt + 1) * 128],
                            idn[:])
                        nc.scalar.copy(vnat[:, kt * 128:(kt + 1) * 128],
                                       ptr[:].bitcast(F32))
                    for qc in range(SQC):
                        ch = b * SQC + qc
                        # mask tiles shared across the 4 heads
                        mtiles = {}
                        for kt in range(SKT):
                            if status[b, kt, qc] == 2:
                                mi = index[(b, kt, qc)]
                                mt_ = mpool.tile([128, 512], F32,
                                                 name=f"mt{b}{qc}{kt}",
                                                 tag=f"mt{kt}")
                                nc.sync.dma_start(
                                    out=mt_[:],
                                    in_=maskP[mi * 128:(mi + 1) * 128, :])
                                mtiles[kt] = mt_
                        kts = [kt for kt in range(SKT)
                               if status[b, kt, qc] != 1]
                        assert kts, "fully-masked softmax row"
                        for h in range(QH):
                            qsl = qT_sb[h][:, boff + qc * 512:
                                           boff + (qc + 1) * 512]
                            pz = ps_x.tile([1, 512], F32, name=f"pz{b}{h}{qc}",
                                           tag="pz")
                            po = ps_o.tile([128, 512], F32,
                                           name=f"po{b}{h}{qc}", tag="po")
                            es = []

                            def drain(j):
                                kt, e = es[j]
                                fst, lst = (j == 0), (j == len(kts) - 1)
                                nc.tensor.matmul(
                                    pz[:], onesb[:], e[:], start=fst,
                                    stop=lst, skip_group_check=True)
                                nc.tensor.matmul(
                                    po[:],
                                    vnat[:, kt * 128:(kt + 1) * 128], e[:],
                                    start=fst, stop=lst,
                                    skip_group_check=True)

                            for i, kt in enumerate(kts):
                                ps_ = ps_s.tile([128, 512], F32,
                                                name=f"s{b}{h}{qc}{kt}",
                                                tag="ps")
                                nc.tensor.matmul(
                                    ps_[:],
                                    kT_sb[:, boff + kt * 128:
                                          boff + (kt + 1) * 128],
                                    qsl, start=True, stop=True,
                                    skip_group_check=True)
                                e = epool.tile([128, 512], BF16,
                                               name=f"e{b}{h}{qc}{kt}",
                                               tag="e")
                                if kt in mtiles:
                                    # scores*sqk + mask, then exp
                                    nc.vector.scalar_tensor_tensor(
                                        ps_[:], ps_[:], BSQK, mtiles[kt][:],
                                        ALU.mult, ALU.add)
                                    nc.scalar.activation(e[:], ps_[:],
                                                         ACTF.Exp)
                                else:
                                    nc.scalar.activation(e[:], ps_[:],
                                                         ACTF.Exp, scale=BSQK)
                                es.append((kt, e))
                                if i >= LOOK:
                                    drain(i - LOOK)
                            for j in range(max(0, len(kts) - LOOK),
                                           len(kts)):
                                drain(j)
                            # zr = gamma_v / z, broadcast to 128 partitions
                            zr = zpool.tile([1, 512], F32,
                                            name=f"zr{b}{h}{qc}", tag="zr")
                            nc.vector.reciprocal(zr[:], pz[:])
                            nc.vector.tensor_scalar(zr[:], zr[:],
                                                    grow[0:1, 4:5], None,
                                                    ALU.mult)
                            zb = zpool.tile([128, 512], F32,
                                            name=f"zb{b}{h}{qc}", tag="zb")
                            nc.gpsimd.partition_broadcast(zb[:], zr[:])
                            ao = aop.tile([128, 512], BF16,
                                          name=f"ao{b}{h}{qc}", tag="ao")
                            nc.vector.tensor_mul(ao[:], po[:], zb[:])
                            nc.sync.dma_start(
                                out=agin[ch][h * 128:(h + 1) * 128, :],
                                in_=ao[:])
                        nc.gpsimd.collective_compute(
                            "AllGather", ALU.bypass, replica_groups=RG,
                            ins=[agin[ch][:].opt()],
                            outs=[agout[ch][:].opt()])

            # ---- phase 3: o_proj in bf16 (out in natural [token, m]) ----
            with tc.tile_pool(name="a3", bufs=1) as a3, \
                 tc.tile_pool(name="o3", bufs=2) as o3, \
                 tc.tile_pool(name="p3", bufs=2, space="PSUM") as p3:
                for ch in range(NTC):
                    ats = []
                    for kt in range(NKT):
                        at = a3.tile([128, 512], BF16, name=f"at{ch}_{kt}",
                                     tag=f"at{kt}")
                        nc.sync.dma_start(
                            out=at[:],
                            in_=agout[ch][kt * 128:(kt + 1) * 128, :])
                        ats.append(at)
                    for tt in range(4):
                        pout = p3.tile([128, 512], F32, name=f"po3_{ch}{tt}",
                                       tag="pout")
                        for kt in range(NKT):
                            nc.tensor.matmul(
                                pout[:],
                                ats[kt][:, tt * 128:(tt + 1) * 128],
                                wo_sb[:, kt * MSH:(kt + 1) * MSH],
                                start=(kt == 0), stop=(kt == NKT - 1),
                                skip_group_check=True)
                        osb = o3.tile([128, 512], F32, name=f"osb{ch}{tt}",
                                      tag="osb")
                        nc.scalar.activation(osb[:], pout[:], ACTF.Copy,
                                             scale=BGO)
                        nc.sync.dma_start(
                            out=outN[ch * 512 + tt * 128:
                                     ch * 512 + (tt + 1) * 128, :],
                            in_=osb[:])
            w3.release()
            qkvp.release()

    nc.compile()
    return nc


def _pretile(wshard):
    """[OUT, H] torch-layout shard -> [128, NKT*OUT] tiled transpose."""
    out_dim = wshard.shape[0]
    wt = np.ascontiguousarray(wshard.T)               # (H, out)
    return np.ascontiguousarray(
        wt.reshape(NKT, 128, out_dim).transpose(1, 0, 2).reshape(
            128, NKT * out_dim))


def kernel(hidden_states, Wq, Wk, Wv, Wo, attention_mask, position_ids):
    from concourse.bass_utils import run_bass_kernel_spmd
    from concourse.bass_interp import get_hw_module

    hs = np.ascontiguousarray(np.asarray(hidden_states, dtype=np.float32))
    Wq = np.asarray(Wq, dtype=np.float32)
    Wk = np.asarray(Wk, dtype=np.float32)
    Wv = np.asarray(Wv, dtype=np.float32)
    Wo = np.asarray(Wo, dtype=np.float32)
    mask = np.asarray(attention_mask, dtype=np.float32)
    posf = np.ascontiguousarray(
        np.asarray(position_ids).reshape(1, T).astype(np.float32))

    status, index, packed = _classify_mask(mask)
    n_packed = packed.shape[0] // 128

    key = (status.tobytes(), n_packed)
    if key not in _cache:
        nc = _build(status, index, n_packed)
        nc.m = get_hw_module(nc.m)
        _cache[key] = nc
    nc = _cache[key]

    xT = np.ascontiguousarray(hs.reshape(T, H).T)
    in_maps = []
    for c in range(NCORES):
        in_maps.append({
            "xT": xT,
            "wqt": _pretile(Wq[c * QH * HD:(c + 1) * QH * HD, :]),
            "wkt": _pretile(Wk[c * HD:(c + 1) * HD, :]),
            "wvt": _pretile(Wv[c * HD:(c + 1) * HD, :]),
            "wot": _pretile(Wo[c * MSH:(c + 1) * MSH, :]),
            "maskP": packed,
            "pos": posf,
        })
    res = run_bass_kernel_spmd(nc, in_maps, core_ids=list(range(NCORES)),
                               trace=bool(os.environ.get("BITNET_TRACE")))
    global last_exec_time_ns
    last_exec_time_ns = res.exec_time_ns
    out = np.concatenate(
        [res.results[c]["outN"] for c in range(NCORES)], axis=1)  # (T, H)
    return np.ascontiguousarray(out).reshape(B, S, H).astype(np.float32)


# revision 25
# speedup vs baseline: 1.2727x; 1.0071x over previous
"""BitNet attention block on 8 TRN2 NeuronCores (tensor-parallel over heads).

Self-contained: kernel(**inputs) takes full inputs, shards internally,
runs one SPMD Bass program on cores 0-7, reassembles the full output.

Sharding: core c owns Q heads [4c,4c+4), KV head c, o_proj output dims
[512c, 512c+512). Attention is fully local per core. Cross-core comms:
two tiny AllReduces for the BitNet absmean gammas and four chunked
bf16 AllGathers (one per (batch, 512-token q-chunk)) of the attention
output so each core can compute its o_proj output-dim shard.

Key structure (vs a naive phase-serial version):
- Weights arrive host-pre-tiled as [128, NKT*fw] so the preamble load
  is a handful of big contiguous DMAs straight into SBUF matmul
  storage, abs-reduced from there and ternarized IN PLACE to unscaled
  {-1,0,+1} right after the gamma AllReduce. Weights live in 4-kt
  group tiles so the projection matmuls chase the quantization at
  group granularity (Tile deps are per-tile).
- All gamma scale factors are folded downstream instead of into the
  weights: gamma_q*gamma_k/sqrt(HD) into the softmax exp scale,
  gamma_v into the 1/z correction, gamma_o into the o_proj PSUM copy.
  The ternary weights are exact in every dtype, so o_proj runs bf16.
- Wo is abs-reduced in the preamble, then re-streamed + ternarized to
  bf16 during phase 1 (engines idle then), staged through DRAM, and
  prefetched into SBUF before o_proj needs it.
- Attention is software-pipelined: score matmuls run 3 k-tiles ahead
  of the pz/po accumulation matmuls so the PE never waits on the
  mask-add/exp chain. exp output and V^T are bf16 (post-softmax path,
  exact-enough), enabling fast weight load on the pz/po matmuls.
- GpSimd does only partition broadcasts/reductions, the gamma bounce
  DMAs and collective triggers (its elementwise path is ~8-15x slower
  than DVE). Engine FIFOs are in-order, so emission order keeps every
  queue head runnable (rope tables first, gammas before quant, wo
  chains late in phase 1).
"""
import os
import sys
sys.path.insert(0, "/opt/trn_rl_repo")
import numpy as np

B, S, H = 2, 1024, 4096
NH, NKV, HD = 32, 8, 128
NCORES = 8
T = B * S
QH = NH // NCORES          # 4 q-heads per core
MSH = H // NCORES          # 512 o_proj out-dims per core
THETA = 10000.0
C_MAGIC = 12582912.0       # 1.5 * 2**23: (x + C) - C == round-half-even(x)
TWO_PI = 6.283185307179586
NKT = H // 128             # 32 contraction tiles
NGRP = 8                   # weight quant/load groups
GKT = NKT // NGRP          # 4 kt per group
NTC = T // 512             # 4 token chunks
SKT = S // 128             # 8 score k-tiles per batch
SQC = S // 512             # 2 q-chunks per batch
LOOK = 3                   # attention score lookahead (software pipeline)

_cache = {}
last_exec_time_ns = None


def _classify_mask(mask):
    """Per (b, kt, qc) [128k x 512q] block: 0 zero, 1 masked-out, 2 general.

    A block whose mask values are all <= -1e4 contributes exactly 0 to the
    fp32 softmax (exp underflows to 0.0), so it is skipped entirely.
    """
    status = np.empty((B, SKT, SQC), dtype=np.int8)
    index = {}
    packed = []
    for b in range(B):
        mb = np.asarray(mask[b, 0], dtype=np.float32)   # (q, k)
        for kt in range(SKT):
            for qc in range(SQC):
                blk = mb[qc * 512:(qc + 1) * 512, kt * 128:(kt + 1) * 128]
                if not blk.any():
                    status[b, kt, qc] = 0
                elif (blk <= -1e4).all():
                    status[b, kt, qc] = 1
                else:
                    status[b, kt, qc] = 2
                    index[(b, kt, qc)] = len(packed)
                    packed.append(np.ascontiguousarray(blk.T))  # (128k, 512q)
    if packed:
        packed_arr = np.concatenate(packed, axis=0).astype(np.float32)
    else:
        packed_arr = np.zeros((128, 512), dtype=np.float32)
    return status, index, packed_arr


def _cody_consts():
    c1 = float(np.float32(6.28125))
    r = np.float64(TWO_PI) - c1
    c2 = float(np.float32(r - np.remainder(r, 2.0 ** -24)))
    c3 = float(np.float32(np.float64(TWO_PI) - c1 - float(c2)))
    return c1, c2, c3


def _build(status, index, n_packed):
    from concourse import bacc, tile, mybir, bass_isa

    F32 = mybir.dt.float32
    F32R = mybir.dt.float32r
    BF16 = mybir.dt.bfloat16
    ACTF = mybir.ActivationFunctionType
    ALU = mybir.AluOpType
    X = mybir.AxisListType.X
    RG = [list(range(NCORES))]
    c1, c2, c3 = _cody_consts()
    ISQ = float(1.0 / np.sqrt(HD))

    nc = bacc.Bacc("TRN2", target_bir_lowering=False, debug=False,
                   num_devices=NCORES)

    # weights are host-pre-tiled: wXt[p, kt*fw + c] = W_shard.T[kt*128+p, c]
    xT = nc.dram_tensor("xT", [H, T], F32, kind="ExternalInput")
    wqt = nc.dram_tensor("wqt", [128, NKT * 512], F32, kind="ExternalInput")
    wkt = nc.dram_tensor("wkt", [128, NKT * 128], F32, kind="ExternalInput")
    wvt = nc.dram_tensor("wvt", [128, NKT * 128], F32, kind="ExternalInput")
    wot = nc.dram_tensor("wot", [128, NKT * 512], F32, kind="ExternalInput")
    maskP = nc.dram_tensor("maskP", [n_packed * 128, 512], F32,
                           kind="ExternalInput")
    pos = nc.dram_tensor("pos", [1, T], F32, kind="ExternalInput")
    outN = nc.dram_tensor("outN", [T, MSH], F32, kind="ExternalOutput")

    idn_c = nc.inline_tensor(np.eye(128, dtype=np.float32), name="idn_c")
    onesm_c = nc.inline_tensor(np.ones((1, 128), np.float32), name="onesm_c")
    invf_np = (1.0 / THETA ** (np.arange(0, HD, 2, dtype=np.float32) / HD))
    invf_np = np.concatenate([invf_np, invf_np]).reshape(HD, 1)
    invf_c = nc.inline_tensor(invf_np.astype(np.float32), name="invf_c")

    NQ = float(NH * HD * H)
    NK = float(NKV * HD * H)
    NO = float(H * NH * HD)
    GW = [512 * GKT, 128 * GKT, 128 * GKT]   # group widths q,k,v

    with tile.TileContext(nc) as tc, \
         nc.allow_low_precision(reason="float32r kernel"):
        with tc.tile_pool(name="cpool", bufs=1) as cpool, \
             tc.tile_pool(name="dbounce", bufs=1, space="DRAM") as dbounce:
            # DRAM bounce tiles (tracked by Tile for collective deps)
            arq_in = dbounce.tile([1, 4], F32, name="arq_in")
            arq_out = dbounce.tile([1, 4], F32, name="arq_out",
                                   addr_space="Shared")
            ark_in = dbounce.tile([1, 4], F32, name="ark_in")
            ark_out = dbounce.tile([1, 4], F32, name="ark_out",
                                   addr_space="Shared")
            agin = [dbounce.tile([QH * HD, 512], BF16, name=f"agin{ch}")
                    for ch in range(NTC)]
            agout = [dbounce.tile([H, 512], BF16, name=f"agout{ch}",
                                  addr_space="Shared") for ch in range(NTC)]
            wo_dram = dbounce.tile([128, NKT * 512], BF16, name="wo_dram")

            idn = cpool.tile([128, 128], F32R, name="idn")
            nc.sync.dma_start(out=idn[:], in_=idn_c[:, :].bitcast(F32R))
            onesb = cpool.tile([128, 1], BF16, name="onesb")
            nc.vector.memset(onesb[:], 1.0)
            onesm = cpool.tile([1, 128], F32, name="onesm")
            nc.sync.dma_start(out=onesm[:], in_=onesm_c[:, :])
            invf = cpool.tile([128, 1], F32, name="invf")
            nc.sync.dma_start(out=invf[:], in_=invf_c[:, :])
            cmag = cpool.tile([128, 1], F32, name="cmag")
            nc.vector.memset(cmag[:], C_MAGIC)
            ncmag = cpool.tile([128, 1], F32, name="ncmag")
            nc.vector.memset(ncmag[:], -C_MAGIC)
            bsc = cpool.tile([128, 8], F32, name="bsc")
            grow = cpool.tile([1, 8], F32, name="grow")

            # long-lived storage: qkv outputs, weight group tiles
            qkvp = tc.alloc_tile_pool(name="qkv", bufs=1)
            qT_sb = [qkvp.tile([128, T], F32R, name=f"qT{h}")
                     for h in range(QH)]
            kT_sb = qkvp.tile([128, T], F32R, name="kT_sb")
            vT_sb = qkvp.tile([128, T], F32R, name="vT_sb")
            wbig = tc.alloc_tile_pool(name="wbig", bufs=1)
            wgrp = [[wbig.tile([128, GW[wi]], F32R, name=f"w{wi}_{g}",
                               tag=f"w{wi}_{g}") for g in range(NGRP)]
                    for wi in range(3)]

            def w_f32(wi, g):
                return wgrp[wi][g][:, :].bitcast(F32)

            def w_kt(wi, kt):
                fw = GW[wi] // GKT
                g, j = kt // GKT, kt % GKT
                return wgrp[wi][g][:, j * fw:(j + 1) * fw]

            wsrcs = [wqt, wkt, wvt]

            with tc.tile_pool(name="tab", bufs=1) as tab:
                cos_sb = tab.tile([128, T], F32, name="cos_sb")
                ss_sb = tab.tile([128, T], F32, name="ss_sb")
                # RoPE tables first: independent of weights, runs at t=0
                # (Cody-Waite range reduction + Sin)
                with tc.tile_pool(name="rtab", bufs=2) as rtab, \
                     tc.tile_pool(name="prt", bufs=2, space="PSUM") as prt:
                    for tcn in range(NTC):
                        cs = slice(tcn * 512, (tcn + 1) * 512)
                        pchunk = rtab.tile([1, 512], F32, name=f"pos{tcn}",
                                           tag="pos")
                        nc.sync.dma_start(out=pchunk[:], in_=pos[0:1, cs])
                        pf = prt.tile([128, 512], F32, name=f"pf{tcn}",
                                      tag="pf")
                        nc.tensor.matmul(pf[:], onesm[:], pchunk[:],
                                         start=True, stop=True)
                        f_sb = rtab.tile([128, 512], F32, name=f"f{tcn}",
                                         tag="f")
                        nc.scalar.activation(f_sb[:], pf[:], ACTF.Copy,
                                             scale=invf[:])
                        k_sb = rtab.tile([128, 512], F32, name=f"kk{tcn}",
                                         tag="kk")
                        nc.vector.tensor_scalar(k_sb[:], f_sb[:],
                                                1.0 / TWO_PI, C_MAGIC,
                                                ALU.mult, ALU.add)
                        nc.vector.tensor_scalar(k_sb[:], k_sb[:], C_MAGIC,
                                                None, ALU.subtract)
                        y_sb = rtab.tile([128, 512], F32, name=f"y{tcn}",
                                         tag="y")
                        nc.vector.scalar_tensor_tensor(
                            y_sb[:], k_sb[:], -c1, f_sb[:], ALU.mult, ALU.add)
                        nc.vector.scalar_tensor_tensor(
                            y_sb[:], k_sb[:], -c2, y_sb[:], ALU.mult, ALU.add)
                        nc.vector.scalar_tensor_tensor(
                            y_sb[:], k_sb[:], -c3, y_sb[:], ALU.mult, ALU.add)
                        nc.scalar.activation(ss_sb[0:64, cs], y_sb[0:64, :],
                                             ACTF.Sin, scale=-1.0)
                        nc.scalar.activation(ss_sb[64:128, cs],
                                             y_sb[64:128, :], ACTF.Sin)
                        yc = rtab.tile([128, 512], F32, name=f"yc{tcn}",
                                       tag="yc")
                        nc.vector.tensor_scalar(yc[:], y_sb[:],
                                                float(np.pi / 2), None,
                                                ALU.add)
                        m_sb = rtab.tile([128, 512], F32, name=f"mm{tcn}",
                                         tag="mm")
                        nc.vector.tensor_scalar(m_sb[:], yc[:],
                                                float(np.pi), None, ALU.is_gt)
                        nc.vector.scalar_tensor_tensor(
                            yc[:], m_sb[:], -TWO_PI, yc[:], ALU.mult, ALU.add)
                        nc.scalar.activation(cos_sb[:, cs], yc[:], ACTF.Sin)

                # gamma accumulation state (lives through phase 1: the wo
                # abs-sum pass runs inside the phase-1 loop)
                g4 = cpool.tile([128, 4], F32, name="g4")
                accs = [cpool.tile([128, 16], F32, name=f"acc{wi}")
                        for wi in range(4)]
                gq_sb = cpool.tile([1, 4], F32, name="gq_sb")
                nc.vector.memset(gq_sb[:], 0.0)
                gk_sb = cpool.tile([1, 4], F32, name="gk_sb")
                nc.vector.memset(gk_sb[:], 0.0)
                gsum = cpool.tile([128, 4], F32, name="gsum")
                gtmp = cpool.tile([1, 8], F32, name="gtmp")
                arq_sb = cpool.tile([1, 4], F32, name="arq_sb")
                ark_sb = cpool.tile([1, 4], F32, name="ark_sb")
                engs = [nc.sync, nc.scalar]

                # ---- load q/k/v weights into final storage + |.| sums;
                # each 4-kt group is two half-DMAs on alternating queues
                # (a single dma_start is capped by per-queue bandwidth)
                ei = 0
                for g in range(NGRP):
                    for wi in range(3):
                        hw = GW[wi] // 2
                        for half in range(2):
                            engs[ei % 2].dma_start(
                                out=wgrp[wi][g][:, half * hw:(half + 1) * hw],
                                in_=wsrcs[wi][:, g * GW[wi] + half * hw:
                                              g * GW[wi] + (half + 1) * hw]
                                .bitcast(F32R))
                            ei += 1
                        nc.vector.tensor_reduce(
                            accs[wi][:, g:g + 1], w_f32(wi, g), X,
                            ALU.add, apply_absolute_value=True)
                for wi in range(3):
                    nc.vector.tensor_reduce(g4[:, wi:wi + 1],
                                            accs[wi][:, 0:NGRP], X, ALU.add)

                # cross-partition totals + gamma AllReduce (bounce DMAs on
                # the otherwise-empty gpsimd queue)
                nc.gpsimd.partition_all_reduce(gsum[:, 0:3], g4[:, 0:3],
                                               128, bass_isa.ReduceOp.add)
                nc.vector.tensor_copy(gq_sb[:, 0:3], gsum[0:1, 0:3])
                nc.gpsimd.dma_start(out=arq_in[:], in_=gq_sb[:])
                nc.gpsimd.collective_compute(
                    "AllReduce", ALU.add, replica_groups=RG,
                    ins=[arq_in[:].opt()], outs=[arq_out[:].opt()])
                nc.gpsimd.dma_start(out=arq_sb[:], in_=arq_out[:])

                # gammas: grow = [1/gq, 1/gk, 1/gv, gq*gk/sqrt(d), gv,
                #                 1/go, go]
                nc.vector.tensor_scalar(gtmp[:, 0:1], arq_sb[:, 0:1],
                                        1.0 / NQ, 1e-5, ALU.mult, ALU.add)
                nc.vector.tensor_scalar(gtmp[:, 1:3], arq_sb[:, 1:3],
                                        1.0 / NK, 1e-5, ALU.mult, ALU.add)
                nc.vector.reciprocal(grow[:, 0:3], gtmp[:, 0:3])
                nc.vector.tensor_mul(gtmp[:, 3:4], gtmp[:, 0:1],
                                     gtmp[:, 1:2])
                nc.vector.tensor_scalar(grow[:, 3:4], gtmp[:, 3:4], ISQ,
                                        None, ALU.mult)
                nc.vector.tensor_copy(grow[:, 4:5], gtmp[:, 2:3])  # gv
                nc.gpsimd.partition_broadcast(bsc[:, 0:5], grow[:, 0:5])
                BSQK = bsc[:, 3:4]
                BIO, BGO = bsc[:, 5:6], bsc[:, 6:7]

                # ---- in-place ternary quant to unscaled {-1,0,+1},
                # per 4-kt group; phase-1 matmuls chase this per group.
                # A(q) on ACT + B/C on DVE; k/v A on DVE, B on ACT.
                with tc.tile_pool(name="qscr", bufs=2) as qscr:
                    for g in range(NGRP):
                        for wi in range(3):
                            s = qscr.tile([128, GW[wi]], F32,
                                          name=f"qs_{wi}_{g}",
                                          tag=f"qs{GW[wi]}")
                            inv_ap = bsc[:, wi:wi + 1]
                            # t = w*(1/gamma) + C (exact round-half-even)
                            if wi == 0:
                                nc.scalar.activation(s[:], w_f32(wi, g),
                                                     ACTF.Identity,
                                                     bias=cmag[:],
                                                     scale=inv_ap)
                                nc.vector.tensor_scalar(
                                    s[:], s[:], C_MAGIC, 1.0,
                                    ALU.subtract, ALU.min)
                            else:
                                nc.vector.tensor_scalar(
                                    s[:], w_f32(wi, g), inv_ap, C_MAGIC,
                                    ALU.mult, ALU.add)
                                nc.scalar.activation(s[:], s[:],
                                                     ACTF.Identity,
                                                     bias=ncmag[:])
                                nc.vector.tensor_scalar(
                                    s[:], s[:], 1.0, None, ALU.min)
                            # w_t = clip to [-1, 1]
                            nc.vector.tensor_scalar(
                                wgrp[wi][g][:, :], s[:], -1.0, None,
                                ALU.max)

                # ---- phase 1: QKV projections + RoPE; wo re-streamed and
                # ternarized to bf16 on the side (emitted late so its chain
                # never blocks the rope stream at any queue head)
                with tc.tile_pool(name="xin", bufs=4) as xin, \
                     tc.tile_pool(name="rope", bufs=2) as rope, \
                     tc.tile_pool(name="wo2", bufs=1) as wo2, \
                     tc.tile_pool(name="wop", bufs=2) as wop, \
                     tc.tile_pool(name="p1", bufs=8, space="PSUM") as p1:

                    def wo_pre(g):
                        # wo abs-sum pass, 2 kt per group (16 groups)
                        slf = wop.tile([128, 1024], F32, name=f"wp_{g}",
                                       tag="wo")
                        engs[g % 2].dma_start(
                            out=slf[:], in_=wot[:, g * 1024:(g + 1) * 1024])
                        nc.vector.tensor_reduce(
                            accs[3][:, g:g + 1], slf[:], X, ALU.add,
                            apply_absolute_value=True)

                    def wo_gamma():
                        nc.vector.tensor_reduce(g4[:, 3:4], accs[3][:], X,
                                                ALU.add)
                        nc.gpsimd.partition_all_reduce(
                            gsum[:, 3:4], g4[:, 3:4], 128,
                            bass_isa.ReduceOp.add)
                        nc.vector.tensor_copy(gk_sb[:, 0:1], gsum[0:1, 3:4])
                        nc.gpsimd.dma_start(out=ark_in[:], in_=gk_sb[:])
                        nc.gpsimd.collective_compute(
                            "AllReduce", ALU.add, replica_groups=RG,
                            ins=[ark_in[:].opt()], outs=[ark_out[:].opt()])
                        nc.gpsimd.dma_start(out=ark_sb[:], in_=ark_out[:])
                        nc.vector.tensor_scalar(gtmp[:, 4:5], ark_sb[:, 0:1],
                                                1.0 / NO, 1e-5, ALU.mult,
                                                ALU.add)
                        nc.vector.reciprocal(grow[:, 5:6], gtmp[:, 4:5])
                        nc.vector.tensor_copy(grow[:, 6:7], gtmp[:, 4:5])
                        nc.gpsimd.partition_broadcast(bsc[:, 5:7],
                                                      grow[:, 5:7])

                    def wo_quant(g):
                        w0, w1 = g * 1024, (g + 1) * 1024
                        src = wo2.tile([128, 1024], F32, name=f"wr_{g}",
                                       tag="wr")
                        nc.scalar.dma_start(out=src[:], in_=wot[:, w0:w1])
                        s = wo2.tile([128, 1024], F32, name=f"ws_{g}",
                                     tag="ws")
                        nc.scalar.activation(s[:], src[:], ACTF.Identity,
                                             bias=cmag[:], scale=BIO)
                        nc.vector.tensor_scalar(s[:], s[:], C_MAGIC, 1.0,
                                                ALU.subtract, ALU.min)
                        wt = wo2.tile([128, 1024], BF16, name=f"wt_{g}",
                                      tag="wt")
                        nc.vector.tensor_scalar(wt[:], s[:], -1.0, None,
                                                ALU.max)
                        nc.scalar.dma_start(out=wo_dram[:, w0:w1], in_=wt[:])

                    for tcn in range(NTC):
                        cs = slice(tcn * 512, (tcn + 1) * 512)
                        pq = [p1.tile([128, 512], F32, name=f"pq{tcn}_{h}",
                                      tag="p1") for h in range(QH)]
                        pk = p1.tile([128, 512], F32, name=f"pk{tcn}",
                                     tag="p1")
                        pv = p1.tile([128, 512], F32, name=f"pv{tcn}",
                                     tag="p1")
                        for kt in range(NKT):
                            xt = xin.tile([128, 512], F32R,
                                          name=f"x{tcn}_{kt}", tag="xt")
                            nc.sync.dma_start(
                                out=xt[:],
                                in_=xT[kt * 128:(kt + 1) * 128, cs]
                                .bitcast(F32R))
                            st, sp = (kt == 0), (kt == NKT - 1)
                            wq_kt = w_kt(0, kt)
                            for h in range(QH):
                                nc.tensor.matmul(
                                    pq[h][:],
                                    wq_kt[:, h * 128:(h + 1) * 128],
                                    xt[:], start=st, stop=sp,
                                    skip_group_check=True)
                            nc.tensor.matmul(
                                pk[:], w_kt(1, kt), xt[:], start=st, stop=sp,
                                skip_group_check=True)
                            nc.tensor.matmul(
                                pv[:], w_kt(2, kt), xt[:], start=st, stop=sp,
                                skip_group_check=True)

                        def rope_apply(psrc, dst_ap, tg):
                            m1 = rope.tile([128, 512], F32, name=f"m1{tg}",
                                           tag="m1")
                            nc.vector.tensor_mul(m1[:], psrc[:], cos_sb[:, cs])
                            m2 = rope.tile([128, 512], F32, name=f"m2{tg}",
                                           tag="m2")
                            nc.vector.tensor_mul(m2[0:64, :], psrc[64:128, :],
                                                 ss_sb[0:64, cs])
                            nc.vector.tensor_mul(m2[64:128, :], psrc[0:64, :],
                                                 ss_sb[64:128, cs])
                            nc.vector.tensor_add(dst_ap, m1[:], m2[:])
                        for h in range(QH):
                            rope_apply(pq[h], qT_sb[h][:, cs], f"_{tcn}_{h}")
                        rope_apply(pk, kT_sb[:, cs], f"k_{tcn}")
                        nc.scalar.copy(vT_sb[:, cs], pv[:])
                        if tcn <= 1:
                            for g in range(tcn * 8, tcn * 8 + 8):
                                wo_pre(g)
                            if tcn == 1:
                                wo_gamma()
                        else:
                            for g in range((tcn - 2) * 8, (tcn - 1) * 8):
                                wo_quant(g)

            wbig.release()  # free weight SBUF before attention
            # prefetch quantized bf16 wo back into SBUF during attention
            w3 = tc.alloc_tile_pool(name="w3", bufs=1)
            wo_sb = w3.tile([128, NKT * MSH], BF16, name="wo_sb")
            for g in range(NGRP):
                w0, w1 = g * 512 * GKT, (g + 1) * 512 * GKT
                nc.sync.dma_start(out=wo_sb[:, w0:w1], in_=wo_dram[:, w0:w1])

            # ---- phase 2: attention (chunked bf16 AllGather per (b,qc)).
            # Software-pipelined at head-group granularity: all 8 score
            # matmuls of group g are emitted before the pz/po drains of
            # group g-1, so the drains always find their exp outputs ready
            # and the PE never waits on the mask-add/exp chain.
            with tc.tile_pool(name="vnatp", bufs=2) as vnatp, \
                 tc.tile_pool(name="epool", bufs=16) as epool, \
                 tc.tile_pool(name="mpool", bufs=1) as mpool, \
                 tc.tile_pool(name="aop", bufs=4) as aop, \
                 tc.tile_pool(name="zpool", bufs=2) as zpool, \
                 tc.tile_pool(name="ps_s", bufs=3, space="PSUM") as ps_s, \
                 tc.tile_pool(name="ps_o", bufs=2, space="PSUM") as ps_o, \
                 tc.tile_pool(name="ps_z", bufs=2, space="PSUM") as ps_z, \
                 tc.tile_pool(name="ps_x", bufs=1, space="PSUM") as ps_x:
                vnats, mtile_maps = {}, {}

                def emit_vnat(b):
                    boff = b * S
                    vnat = vnatp.tile([128, S], BF16, name=f"vnat{b}",
                                      tag="vnat")
                    for kt in range(SKT):
                        ptr = ps_x.tile([128, 128], F32R, name=f"ptr{b}_{kt}",
                                        tag="ptr")
                        nc.tensor.transpose(
                            ptr[:],
                            vT_sb[:, boff + kt * 128:boff + (kt + 1) * 128],
                            idn[:])
                        nc.scalar.copy(vnat[:, kt * 128:(kt + 1) * 128],
                                       ptr[:].bitcast(F32))
                    vnats[b] = vnat

                def emit_masks(b, qc):
                    mtiles = {}
                    for kt in range(SKT):
                        if status[b, kt, qc] == 2:
                            mi = index[(b, kt, qc)]
                            mt_ = mpool.tile([128, 512], F32,
                                             name=f"mt{b}{qc}{kt}",
                                             tag=f"mt{kt}")
                            nc.sync.dma_start(
                                out=mt_[:],
                                in_=maskP[mi * 128:(mi + 1) * 128, :])
                            mtiles[kt] = mt_
                    mtile_maps[(b, qc)] = mtiles

                def emit_scores(g):
                    b, qc, h = g["b"], g["qc"], g["h"]
                    boff = b * S
                    mtiles = mtile_maps[(b, qc)]
                    qsl = qT_sb[h][:, boff + qc * 512:boff + (qc + 1) * 512]
                    for kt in g["kts"]:
                        ps_ = ps_s.tile([128, 512], F32,
                                        name=f"s{b}{h}{qc}{kt}", tag="ps")
                        nc.tensor.matmul(
                            ps_[:],
                            kT_sb[:, boff + kt * 128:boff + (kt + 1) * 128],
                            qsl, start=True, stop=True,
                            skip_group_check=True)
                        e = epool.tile([128, 512], BF16,
                                       name=f"e{b}{h}{qc}{kt}", tag="e")
                        if kt in mtiles:
                            # scores*sqk + mask, then exp
                            nc.vector.scalar_tensor_tensor(
                                ps_[:], ps_[:], BSQK, mtiles[kt][:],
                                ALU.mult, ALU.add)
                            nc.scalar.activation(e[:], ps_[:], ACTF.Exp)
                        else:
                            nc.scalar.activation(e[:], ps_[:], ACTF.Exp,
                                                 scale=BSQK)
                        g["es"].append((kt, e))

                def emit_drains(g):
                    b, qc, h, ch = g["b"], g["qc"], g["h"], g["ch"]
                    vnat = vnats[b]
                    pz = ps_z.tile([1, 512], F32, name=f"pz{b}{h}{qc}",
                                   tag="pz")
                    po = ps_o.tile([128, 512], F32, name=f"po{b}{h}{qc}",
                                   tag="po")
                    n = len(g["es"])
                    for j, (kt, e) in enumerate(g["es"]):
                        fst, lst = (j == 0), (j == n - 1)
                        nc.tensor.matmul(pz[:], onesb[:], e[:], start=fst,
                                         stop=lst, skip_group_check=True)
                        nc.tensor.matmul(
                            po[:], vnat[:, kt * 128:(kt + 1) * 128], e[:],
                            start=fst, stop=lst, skip_group_check=True)
                    # zr = gamma_v / z, broadcast to 128 partitions
                    zr = zpool.tile([1, 512], F32, name=f"zr{b}{h}{qc}",
                                    tag="zr")
                    nc.vector.reciprocal(zr[:], pz[:])
                    nc.vector.tensor_scalar(zr[:], zr[:], grow[0:1, 4:5],
                                            None, ALU.mult)
                    zb = zpool.tile([128, 512], F32, name=f"zb{b}{h}{qc}",
                                    tag="zb")
                    nc.gpsimd.partition_broadcast(zb[:], zr[:])
                    ao = aop.tile([128, 512], BF16, name=f"ao{b}{h}{qc}",
                                  tag="ao")
                    nc.vector.tensor_mul(ao[:], po[:], zb[:])
                    nc.sync.dma_start(
                        out=agin[ch][h * 128:(h + 1) * 128, :], in_=ao[:])
                    if h == QH - 1:
                        nc.gpsimd.collective_compute(
                            "AllGather", ALU.bypass, replica_groups=RG,
                            ins=[agin[ch][:].opt()],
                            outs=[agout[ch][:].opt()])

                groups = []
                for b in range(B):
                    for qc in range(SQC):
                        kts = [kt for kt in range(SKT)
                               if status[b, kt, qc] != 1]
                        assert kts, "fully-masked softmax row"
                        for h in range(QH):
                            groups.append({"b": b, "qc": qc, "h": h,
                                           "ch": b * SQC + qc, "kts": kts,
                                           "es": []})
                prev = None
                for g in groups:
                    if g["h"] == 0 and g["qc"] == 0:
                        emit_vnat(g["b"])
                    if g["h"] == 0:
                        emit_masks(g["b"], g["qc"])
                    emit_scores(g)
                    if prev is not None:
                        emit_drains(prev)
                    prev = g
                emit_drains(prev)

            # ---- phase 3: o_proj in bf16 (out in natural [token, m]) ----
            with tc.tile_pool(name="a3", bufs=1) as a3, \
                 tc.tile_pool(name="o3", bufs=2) as o3, \
                 tc.tile_pool(name="p3", bufs=2, space="PSUM") as p3:
                for ch in range(NTC):
                    ats = []
                    for kt in range(NKT):
                        at = a3.tile([128, 512], BF16, name=f"at{ch}_{kt}",
                                     tag=f"at{kt}")
                        nc.sync.dma_start(
                            out=at[:],
                            in_=agout[ch][kt * 128:(kt + 1) * 128, :])
                        ats.append(at)
                    for tt in range(4):
                        pout = p3.tile([128, 512], F32, name=f"po3_{ch}{tt}",
                                       tag="pout")
                        for kt in range(NKT):
                            nc.tensor.matmul(
                                pout[:],
                                ats[kt][:, tt * 128:(tt + 1) * 128],
                                wo_sb[:, kt * MSH:(kt + 1) * MSH],
                                start=(kt == 0), stop=(kt == NKT - 1),
                                skip_group_check=True)
                        osb = o3.tile([128, 512], F32, name=f"osb{ch}{tt}",
                                      tag="osb")
                        nc.scalar.activation(osb[:], pout[:], ACTF.Copy,
                                             scale=BGO)
                        nc.sync.dma_start(
                            out=outN[ch * 512 + tt * 128:
                                     ch * 512 + (tt + 1) * 128, :],
                            in_=osb[:])
            w3.release()
            qkvp.release()

    nc.compile()
    return nc


def _pretile(wshard):
    """[OUT, H] torch-layout shard -> [128, NKT*OUT] tiled transpose."""
    out_dim = wshard.shape[0]
    wt = np.ascontiguousarray(wshard.T)               # (H, out)
    return np.ascontiguousarray(
        wt.reshape(NKT, 128, out_dim).transpose(1, 0, 2).reshape(
            128, NKT * out_dim))


def kernel(hidden_states, Wq, Wk, Wv, Wo, attention_mask, position_ids):
    from concourse.bass_utils import run_bass_kernel_spmd
    from concourse.bass_interp import get_hw_module

    hs = np.ascontiguousarray(np.asarray(hidden_states, dtype=np.float32))
    Wq = np.asarray(Wq, dtype=np.float32)
    Wk = np.asarray(Wk, dtype=np.float32)
    Wv = np.asarray(Wv, dtype=np.float32)
    Wo = np.asarray(Wo, dtype=np.float32)
    mask = np.asarray(attention_mask, dtype=np.float32)
    posf = np.ascontiguousarray(
        np.asarray(position_ids).reshape(1, T).astype(np.float32))

    status, index, packed = _classify_mask(mask)
    n_packed = packed.shape[0] // 128

    key = (status.tobytes(), n_packed)
    if key not in _cache:
        nc = _build(status, index, n_packed)
        nc.m = get_hw_module(nc.m)
        _cache[key] = nc
    nc = _cache[key]

    xT = np.ascontiguousarray(hs.reshape(T, H).T)
    in_maps = []
    for c in range(NCORES):
        in_maps.append({
            "xT": xT,
            "wqt": _pretile(Wq[c * QH * HD:(c + 1) * QH * HD, :]),
            "wkt": _pretile(Wk[c * HD:(c + 1) * HD, :]),
            "wvt": _pretile(Wv[c * HD:(c + 1) * HD, :]),
            "wot": _pretile(Wo[c * MSH:(c + 1) * MSH, :]),
            "maskP": packed,
            "pos": posf,
        })
    res = run_bass_kernel_spmd(nc, in_maps, core_ids=list(range(NCORES)),
                               trace=bool(os.environ.get("BITNET_TRACE")))
    global last_exec_time_ns
    last_exec_time_ns = res.exec_time_ns
    out = np.concatenate(
        [res.results[c]["outN"] for c in range(NCORES)], axis=1)  # (T, H)
    return np.ascontiguousarray(out).reshape(B, S, H).astype(np.float32)


# revision 34
# speedup vs baseline: 1.3356x; 1.0494x over previous
"""BitNet attention block on 8 TRN2 NeuronCores (tensor-parallel over heads).

Self-contained: kernel(**inputs) takes full inputs, shards internally,
runs one SPMD Bass program on cores 0-7, reassembles the full output.

Sharding: core c owns Q heads [4c,4c+4), KV head c, o_proj output dims
[512c, 512c+512). Attention is fully local per core. Cross-core comms:
two tiny AllReduces for the BitNet absmean gammas and four chunked
bf16 AllGathers (one per (batch, 512-token q-chunk)) of the attention
output so each core can compute its o_proj output-dim shard.

Key structure (vs a naive phase-serial version):
- Weights arrive host-pre-tiled as [128, NKT*fw] so the preamble load
  is a handful of big contiguous DMAs straight into SBUF matmul
  storage, abs-reduced from there and ternarized IN PLACE to unscaled
  {-1,0,+1} right after the gamma AllReduce. Weights live in 4-kt
  group tiles so the projection matmuls chase the quantization at
  group granularity (Tile deps are per-tile).
- All gamma scale factors are folded downstream instead of into the
  weights: gamma_q*gamma_k/sqrt(HD) into the softmax exp scale,
  gamma_v into the 1/z correction, gamma_o into the o_proj PSUM copy.
  The ternary weights are exact in every dtype, so o_proj runs bf16.
- Wo is abs-reduced in the preamble, then re-streamed + ternarized to
  bf16 during phase 1 (engines idle then), staged through DRAM, and
  prefetched into SBUF before o_proj needs it.
- Attention is software-pipelined: score matmuls run 3 k-tiles ahead
  of the pz/po accumulation matmuls so the PE never waits on the
  mask-add/exp chain. exp output and V^T are bf16 (post-softmax path,
  exact-enough), enabling fast weight load on the pz/po matmuls.
- GpSimd does only partition broadcasts/reductions, the gamma bounce
  DMAs and collective triggers (its elementwise path is ~8-15x slower
  than DVE). Engine FIFOs are in-order, so emission order keeps every
  queue head runnable (rope tables first, gammas before quant, wo
  chains late in phase 1).
"""
import os
import sys
sys.path.insert(0, "/opt/trn_rl_repo")
import numpy as np

B, S, H = 2, 1024, 4096
NH, NKV, HD = 32, 8, 128
NCORES = 8
T = B * S
QH = NH // NCORES          # 4 q-heads per core
MSH = H // NCORES          # 512 o_proj out-dims per core
THETA = 10000.0
C_MAGIC = 12582912.0       # 1.5 * 2**23: (x + C) - C == round-half-even(x)
TWO_PI = 6.283185307179586
NKT = H // 128             # 32 contraction tiles
NGRP = 8                   # weight quant/load groups
GKT = NKT // NGRP          # 4 kt per group
NTC = T // 512             # 4 token chunks
SKT = S // 128             # 8 score k-tiles per batch
SQC = S // 512             # 2 q-chunks per batch
LOOK = 3                   # attention score lookahead (software pipeline)

_cache = {}
last_exec_time_ns = None


def _classify_mask(mask):
    """Per (b, kt, qc) [128k x 512q] block: 0 zero, 1 masked-out, 2 general.

    A block whose mask values are all <= -1e4 contributes exactly 0 to the
    fp32 softmax (exp underflows to 0.0), so it is skipped entirely.
    """
    status = np.empty((B, SKT, SQC), dtype=np.int8)
    index = {}
    packed = []
    for b in range(B):
        mb = np.asarray(mask[b, 0], dtype=np.float32)   # (q, k)
        for kt in range(SKT):
            for qc in range(SQC):
                blk = mb[qc * 512:(qc + 1) * 512, kt * 128:(kt + 1) * 128]
                if not blk.any():
                    status[b, kt, qc] = 0
                elif (blk <= -1e4).all():
                    status[b, kt, qc] = 1
                else:
                    status[b, kt, qc] = 2
                    index[(b, kt, qc)] = len(packed)
                    packed.append(np.ascontiguousarray(blk.T))  # (128k, 512q)
    if packed:
        packed_arr = np.concatenate(packed, axis=0).astype(np.float32)
    else:
        packed_arr = np.zeros((128, 512), dtype=np.float32)
    return status, index, packed_arr


def _cody_consts():
    c1 = float(np.float32(6.28125))
    r = np.float64(TWO_PI) - c1
    c2 = float(np.float32(r - np.remainder(r, 2.0 ** -24)))
    c3 = float(np.float32(np.float64(TWO_PI) - c1 - float(c2)))
    return c1, c2, c3


def _build(status, index, n_packed):
    from concourse import bacc, tile, mybir, bass_isa

    F32 = mybir.dt.float32
    F32R = mybir.dt.float32r
    BF16 = mybir.dt.bfloat16
    ACTF = mybir.ActivationFunctionType
    ALU = mybir.AluOpType
    X = mybir.AxisListType.X
    RG = [list(range(NCORES))]
    c1, c2, c3 = _cody_consts()
    ISQ = float(1.0 / np.sqrt(HD))

    nc = bacc.Bacc("TRN2", target_bir_lowering=False, debug=False,
                   num_devices=NCORES)

    # weights are host-pre-tiled: wXt[p, kt*fw + c] = W_shard.T[kt*128+p, c]
    xT = nc.dram_tensor("xT", [H, T], BF16, kind="ExternalInput")
    wqt = nc.dram_tensor("wqt", [128, NKT * 512], F32, kind="ExternalInput")
    wkt = nc.dram_tensor("wkt", [128, NKT * 128], F32, kind="ExternalInput")
    wvt = nc.dram_tensor("wvt", [128, NKT * 128], F32, kind="ExternalInput")
    wot = nc.dram_tensor("wot", [128, NKT * 512], F32, kind="ExternalInput")
    maskP = nc.dram_tensor("maskP", [n_packed * 128, 512], F32,
                           kind="ExternalInput")
    pos = nc.dram_tensor("pos", [1, T], F32, kind="ExternalInput")
    outN = nc.dram_tensor("outN", [T, MSH], F32, kind="ExternalOutput")

    idn_c = nc.inline_tensor(np.eye(128, dtype=np.float32), name="idn_c")
    onesm_c = nc.inline_tensor(np.ones((1, 128), np.float32), name="onesm_c")
    invf_np = (1.0 / THETA ** (np.arange(0, HD, 2, dtype=np.float32) / HD))
    invf_np = np.concatenate([invf_np, invf_np]).reshape(HD, 1)
    invf_c = nc.inline_tensor(invf_np.astype(np.float32), name="invf_c")

    NQ = float(NH * HD * H)
    NK = float(NKV * HD * H)
    NO = float(H * NH * HD)
    GW = [512 * GKT, 128 * GKT, 128 * GKT]   # group widths q,k,v

    with tile.TileContext(nc) as tc, \
         nc.allow_low_precision(reason="float32r kernel"):
        with tc.tile_pool(name="cpool", bufs=1) as cpool, \
             tc.tile_pool(name="dbounce", bufs=1, space="DRAM") as dbounce:
            # DRAM bounce tiles (tracked by Tile for collective deps)
            arq_in = dbounce.tile([1, 4], F32, name="arq_in")
            arq_out = dbounce.tile([1, 4], F32, name="arq_out",
                                   addr_space="Shared")
            ark_in = dbounce.tile([1, 4], F32, name="ark_in")
            ark_out = dbounce.tile([1, 4], F32, name="ark_out",
                                   addr_space="Shared")
            agin = [dbounce.tile([QH * HD, 512], BF16, name=f"agin{ch}")
                    for ch in range(NTC)]
            agout = [dbounce.tile([H, 512], BF16, name=f"agout{ch}",
                                  addr_space="Shared") for ch in range(NTC)]
            wo_dram = dbounce.tile([128, NKT * 512], BF16, name="wo_dram")

            idn_f = cpool.tile([128, 128], F32, name="idn_f")
            nc.sync.dma_start(out=idn_f[:], in_=idn_c[:, :])
            idn = cpool.tile([128, 128], BF16, name="idn")
            nc.vector.tensor_copy(idn[:], idn_f[:])
            onesb = cpool.tile([128, 1], BF16, name="onesb")
            nc.vector.memset(onesb[:], 1.0)
            onesm = cpool.tile([1, 128], F32, name="onesm")
            nc.sync.dma_start(out=onesm[:], in_=onesm_c[:, :])
            invf = cpool.tile([128, 1], F32, name="invf")
            nc.sync.dma_start(out=invf[:], in_=invf_c[:, :])
            cmag = cpool.tile([128, 1], F32, name="cmag")
            nc.vector.memset(cmag[:], C_MAGIC)
            ncmag = cpool.tile([128, 1], F32, name="ncmag")
            nc.vector.memset(ncmag[:], -C_MAGIC)
            bsc = cpool.tile([128, 8], F32, name="bsc")
            grow = cpool.tile([1, 8], F32, name="grow")

            # long-lived storage: qkv outputs (bf16) and ternary bf16 weights
            qkvp = tc.alloc_tile_pool(name="qkv", bufs=1)
            qT_sb = [qkvp.tile([128, T], BF16, name=f"qT{h}")
                     for h in range(QH)]
            kT_sb = qkvp.tile([128, T], BF16, name="kT_sb")
            vT_sb = qkvp.tile([128, T], BF16, name="vT_sb")
            wbf = tc.alloc_tile_pool(name="wbf", bufs=1)
            wgrb = [[wbf.tile([128, GW[wi]], BF16, name=f"wb{wi}_{g}",
                              tag=f"wb{wi}_{g}") for g in range(NGRP)]
                    for wi in range(3)]

            def w_kt(wi, kt):
                fw = GW[wi] // GKT
                g, j = kt // GKT, kt % GKT
                return wgrb[wi][g][:, j * fw:(j + 1) * fw]

            wsrcs = [wqt, wkt, wvt]

            with tc.tile_pool(name="tab", bufs=1) as tab:
                # raw f32 weights: staging only, freed before phase 1 pools
                wbig = tc.alloc_tile_pool(name="wbig", bufs=1)
                wgrp = [[wbig.tile([128, GW[wi]], F32, name=f"w{wi}_{g}",
                                   tag=f"w{wi}_{g}") for g in range(NGRP)]
                        for wi in range(3)]

                def w_f32(wi, g):
                    return wgrp[wi][g][:, :]
                cos_sb = tab.tile([128, T], F32, name="cos_sb")
                ss_sb = tab.tile([128, T], F32, name="ss_sb")
                # RoPE tables first: independent of weights, runs at t=0
                # (Cody-Waite range reduction + Sin)
                with tc.tile_pool(name="rtab", bufs=1) as rtab, \
                     tc.tile_pool(name="prt", bufs=2, space="PSUM") as prt:
                    for tcn in range(NTC):
                        cs = slice(tcn * 512, (tcn + 1) * 512)
                        pchunk = rtab.tile([1, 512], F32, name=f"pos{tcn}",
                                           tag="pos")
                        nc.sync.dma_start(out=pchunk[:], in_=pos[0:1, cs])
                        pf = prt.tile([128, 512], F32, name=f"pf{tcn}",
                                      tag="pf")
                        nc.tensor.matmul(pf[:], onesm[:], pchunk[:],
                                         start=True, stop=True)
                        f_sb = rtab.tile([128, 512], F32, name=f"f{tcn}",
                                         tag="f")
                        nc.scalar.activation(f_sb[:], pf[:], ACTF.Copy,
                                             scale=invf[:])
                        k_sb = rtab.tile([128, 512], F32, name=f"kk{tcn}",
                                         tag="kk")
                        nc.vector.tensor_scalar(k_sb[:], f_sb[:],
                                                1.0 / TWO_PI, C_MAGIC,
                                                ALU.mult, ALU.add)
                        nc.vector.tensor_scalar(k_sb[:], k_sb[:], C_MAGIC,
                                                None, ALU.subtract)
                        y_sb = rtab.tile([128, 512], F32, name=f"y{tcn}",
                                         tag="y")
                        nc.vector.scalar_tensor_tensor(
                            y_sb[:], k_sb[:], -c1, f_sb[:], ALU.mult, ALU.add)
                        nc.vector.scalar_tensor_tensor(
                            y_sb[:], k_sb[:], -c2, y_sb[:], ALU.mult, ALU.add)
                        nc.vector.scalar_tensor_tensor(
                            y_sb[:], k_sb[:], -c3, y_sb[:], ALU.mult, ALU.add)
                        nc.scalar.activation(ss_sb[0:64, cs], y_sb[0:64, :],
                                             ACTF.Sin, scale=-1.0)
                        nc.scalar.activation(ss_sb[64:128, cs],
                                             y_sb[64:128, :], ACTF.Sin)
                        yc = rtab.tile([128, 512], F32, name=f"yc{tcn}",
                                       tag="yc")
                        nc.vector.tensor_scalar(yc[:], y_sb[:],
                                                float(np.pi / 2), None,
                                                ALU.add)
                        m_sb = rtab.tile([128, 512], F32, name=f"mm{tcn}",
                                         tag="mm")
                        nc.vector.tensor_scalar(m_sb[:], yc[:],
                                                float(np.pi), None, ALU.is_gt)
                        nc.vector.scalar_tensor_tensor(
                            yc[:], m_sb[:], -TWO_PI, yc[:], ALU.mult, ALU.add)
                        nc.scalar.activation(cos_sb[:, cs], yc[:], ACTF.Sin)

                # gamma accumulation state (lives through phase 1: the wo
                # abs-sum pass runs inside the phase-1 loop)
                g4 = cpool.tile([128, 4], F32, name="g4")
                accs = [cpool.tile([128, 16], F32, name=f"acc{wi}")
                        for wi in range(4)]
                gq_sb = cpool.tile([1, 4], F32, name="gq_sb")
                nc.vector.memset(gq_sb[:], 0.0)
                gk_sb = cpool.tile([1, 4], F32, name="gk_sb")
                nc.vector.memset(gk_sb[:], 0.0)
                gsum = cpool.tile([128, 4], F32, name="gsum")
                gtmp = cpool.tile([1, 8], F32, name="gtmp")
                arq_sb = cpool.tile([1, 4], F32, name="arq_sb")
                ark_sb = cpool.tile([1, 4], F32, name="ark_sb")
                engs = [nc.sync, nc.scalar]

                # ---- load q/k/v weights into final storage + |.| sums;
                # each 4-kt group is two half-DMAs on alternating queues
                # (a single dma_start is capped by per-queue bandwidth)
                ei = 0
                for g in range(NGRP):
                    for wi in range(3):
                        hw = GW[wi] // 2
                        for half in range(2):
                            engs[ei % 2].dma_start(
                                out=wgrp[wi][g][:, half * hw:(half + 1) * hw],
                                in_=wsrcs[wi][:, g * GW[wi] + half * hw:
                                              g * GW[wi] + (half + 1) * hw])
                            ei += 1
                        nc.vector.tensor_reduce(
                            accs[wi][:, g:g + 1], w_f32(wi, g), X,
                            ALU.add, apply_absolute_value=True)
                for wi in range(3):
                    nc.vector.tensor_reduce(g4[:, wi:wi + 1],
                                            accs[wi][:, 0:NGRP], X, ALU.add)

                # cross-partition totals + gamma AllReduce (bounce DMAs on
                # the otherwise-empty gpsimd queue)
                nc.gpsimd.partition_all_reduce(gsum[:, 0:3], g4[:, 0:3],
                                               128, bass_isa.ReduceOp.add)
                nc.vector.tensor_copy(gq_sb[:, 0:3], gsum[0:1, 0:3])
                nc.gpsimd.dma_start(out=arq_in[:], in_=gq_sb[:])
                nc.gpsimd.collective_compute(
                    "AllReduce", ALU.add, replica_groups=RG,
                    ins=[arq_in[:].opt()], outs=[arq_out[:].opt()])
                nc.gpsimd.dma_start(out=arq_sb[:], in_=arq_out[:])

                # gammas: grow = [1/gq, 1/gk, 1/gv, gq*gk/sqrt(d), gv,
                #                 1/go, go]
                nc.vector.tensor_scalar(gtmp[:, 0:1], arq_sb[:, 0:1],
                                        1.0 / NQ, 1e-5, ALU.mult, ALU.add)
                nc.vector.tensor_scalar(gtmp[:, 1:3], arq_sb[:, 1:3],
                                        1.0 / NK, 1e-5, ALU.mult, ALU.add)
                nc.vector.reciprocal(grow[:, 0:3], gtmp[:, 0:3])
                nc.vector.tensor_mul(gtmp[:, 3:4], gtmp[:, 0:1],
                                     gtmp[:, 1:2])
                nc.vector.tensor_scalar(grow[:, 3:4], gtmp[:, 3:4], ISQ,
                                        None, ALU.mult)
                nc.vector.tensor_copy(grow[:, 4:5], gtmp[:, 2:3])  # gv
                nc.gpsimd.partition_broadcast(bsc[:, 0:5], grow[:, 0:5])
                BSQK = bsc[:, 3:4]
                BIO, BGO = bsc[:, 5:6], bsc[:, 6:7]

                # ---- in-place ternary quant to unscaled {-1,0,+1},
                # per 4-kt group; phase-1 matmuls chase this per group.
                # A(q) on ACT + B/C on DVE; k/v A on DVE, B on ACT.
                with tc.tile_pool(name="qscr", bufs=2) as qscr:
                    for g in range(NGRP):
                        for wi in range(3):
                            s = qscr.tile([128, GW[wi]], F32,
                                          name=f"qs_{wi}_{g}",
                                          tag=f"qs{GW[wi]}")
                            inv_ap = bsc[:, wi:wi + 1]
                            # t = w*(1/gamma) + C (exact round-half-even)
                            if wi == 0:
                                nc.scalar.activation(s[:], w_f32(wi, g),
                                                     ACTF.Identity,
                                                     bias=cmag[:],
                                                     scale=inv_ap)
                                nc.vector.tensor_scalar(
                                    s[:], s[:], C_MAGIC, 1.0,
                                    ALU.subtract, ALU.min)
                            else:
                                nc.vector.tensor_scalar(
                                    s[:], w_f32(wi, g), inv_ap, C_MAGIC,
                                    ALU.mult, ALU.add)
                                nc.scalar.activation(s[:], s[:],
                                                     ACTF.Identity,
                                                     bias=ncmag[:])
                                nc.vector.tensor_scalar(
                                    s[:], s[:], 1.0, None, ALU.min)
                            # w_t = clip to [-1, 1], ternary exact in bf16
                            nc.vector.tensor_scalar(
                                wgrb[wi][g][:, :], s[:], -1.0, None,
                                ALU.max)
                wbig.release()  # raw f32 weights dead after quant

                # ---- phase 1: QKV projections + RoPE; wo re-streamed and
                # ternarized to bf16 on the side (emitted late so its chain
                # never blocks the rope stream at any queue head)
                with tc.tile_pool(name="xin", bufs=4) as xin, \
                     tc.tile_pool(name="rope", bufs=2) as rope, \
                     tc.tile_pool(name="wo2", bufs=1) as wo2, \
                     tc.tile_pool(name="wop", bufs=2) as wop, \
                     tc.tile_pool(name="p1", bufs=8, space="PSUM") as p1:

                    def wo_pre(g):
                        # wo abs-sum pass, 2 kt per group (16 groups)
                        slf = wop.tile([128, 1024], F32, name=f"wp_{g}",
                                       tag="wo")
                        engs[g % 2].dma_start(
                            out=slf[:], in_=wot[:, g * 1024:(g + 1) * 1024])
                        nc.vector.tensor_reduce(
                            accs[3][:, g:g + 1], slf[:], X, ALU.add,
                            apply_absolute_value=True)

                    def wo_gamma():
                        nc.vector.tensor_reduce(g4[:, 3:4], accs[3][:], X,
                                                ALU.add)
                        nc.gpsimd.partition_all_reduce(
                            gsum[:, 3:4], g4[:, 3:4], 128,
                            bass_isa.ReduceOp.add)
                        nc.vector.tensor_copy(gk_sb[:, 0:1], gsum[0:1, 3:4])
                        nc.gpsimd.dma_start(out=ark_in[:], in_=gk_sb[:])
                        nc.gpsimd.collective_compute(
                            "AllReduce", ALU.add, replica_groups=RG,
                            ins=[ark_in[:].opt()], outs=[ark_out[:].opt()])
                        nc.gpsimd.dma_start(out=ark_sb[:], in_=ark_out[:])
                        nc.vector.tensor_scalar(gtmp[:, 4:5], ark_sb[:, 0:1],
                                                1.0 / NO, 1e-5, ALU.mult,
                                                ALU.add)
                        nc.vector.reciprocal(grow[:, 5:6], gtmp[:, 4:5])
                        nc.vector.tensor_copy(grow[:, 6:7], gtmp[:, 4:5])
                        nc.gpsimd.partition_broadcast(bsc[:, 5:7],
                                                      grow[:, 5:7])

                    def wo_quant(g):
                        w0, w1 = g * 1024, (g + 1) * 1024
                        src = wo2.tile([128, 1024], F32, name=f"wr_{g}",
                                       tag="wr")
                        nc.scalar.dma_start(out=src[:], in_=wot[:, w0:w1])
                        s = wo2.tile([128, 1024], F32, name=f"ws_{g}",
                                     tag="ws")
                        nc.scalar.activation(s[:], src[:], ACTF.Identity,
                                             bias=cmag[:], scale=BIO)
                        nc.vector.tensor_scalar(s[:], s[:], C_MAGIC, 1.0,
                                                ALU.subtract, ALU.min)
                        wt = wo2.tile([128, 1024], BF16, name=f"wt_{g}",
                                      tag="wt")
                        nc.vector.tensor_scalar(wt[:], s[:], -1.0, None,
                                                ALU.max)
                        nc.scalar.dma_start(out=wo_dram[:, w0:w1], in_=wt[:])

                    for tcn in range(NTC):
                        cs = slice(tcn * 512, (tcn + 1) * 512)
                        pq = [p1.tile([128, 512], F32, name=f"pq{tcn}_{h}",
                                      tag="p1") for h in range(QH)]
                        pk = p1.tile([128, 512], F32, name=f"pk{tcn}",
                                     tag="p1")
                        pv = p1.tile([128, 512], F32, name=f"pv{tcn}",
                                     tag="p1")
                        for kt in range(NKT):
                            xt = xin.tile([128, 512], BF16,
                                          name=f"x{tcn}_{kt}", tag="xt")
                            nc.sync.dma_start(
                                out=xt[:],
                                in_=xT[kt * 128:(kt + 1) * 128, cs])
                            st, sp = (kt == 0), (kt == NKT - 1)
                            wq_kt = w_kt(0, kt)
                            for h in range(QH):
                                nc.tensor.matmul(
                                    pq[h][:],
                                    wq_kt[:, h * 128:(h + 1) * 128],
                                    xt[:], start=st, stop=sp,
                                    skip_group_check=True)
                            nc.tensor.matmul(
                                pk[:], w_kt(1, kt), xt[:], start=st, stop=sp,
                                skip_group_check=True)
                            nc.tensor.matmul(
                                pv[:], w_kt(2, kt), xt[:], start=st, stop=sp,
                                skip_group_check=True)

                        def rope_apply(psrc, dst_ap, tg):
                            m1 = rope.tile([128, 512], F32, name=f"m1{tg}",
                                           tag="m1")
                            nc.vector.tensor_mul(m1[:], psrc[:], cos_sb[:, cs])
                            m2 = rope.tile([128, 512], F32, name=f"m2{tg}",
                                           tag="m2")
                            nc.vector.tensor_mul(m2[0:64, :], psrc[64:128, :],
                                                 ss_sb[0:64, cs])
                            nc.vector.tensor_mul(m2[64:128, :], psrc[0:64, :],
                                                 ss_sb[64:128, cs])
                            nc.vector.tensor_add(dst_ap, m1[:], m2[:])
                        for h in range(QH):
                            rope_apply(pq[h], qT_sb[h][:, cs], f"_{tcn}_{h}")
                        rope_apply(pk, kT_sb[:, cs], f"k_{tcn}")
                        nc.scalar.copy(vT_sb[:, cs], pv[:])
                        if tcn <= 1:
                            for g in range(tcn * 8, tcn * 8 + 8):
                                wo_pre(g)
                            if tcn == 1:
                                wo_gamma()
                        else:
                            for g in range((tcn - 2) * 8, (tcn - 1) * 8):
                                wo_quant(g)

            wbf.release()  # free ternary weight SBUF before attention
            # prefetch quantized bf16 wo back into SBUF during attention
            w3 = tc.alloc_tile_pool(name="w3", bufs=1)
            wo_sb = w3.tile([128, NKT * MSH], BF16, name="wo_sb")
            for g in range(NGRP):
                w0, w1 = g * 512 * GKT, (g + 1) * 512 * GKT
                nc.sync.dma_start(out=wo_sb[:, w0:w1], in_=wo_dram[:, w0:w1])

            # ---- phase 2: attention (chunked bf16 AllGather per (b,qc)).
            # Software-pipelined at head-group granularity: all 8 score
            # matmuls of group g are emitted before the pz/po drains of
            # group g-1, so the drains always find their exp outputs ready
            # and the PE never waits on the mask-add/exp chain.
            with tc.tile_pool(name="vnatp", bufs=2) as vnatp, \
                 tc.tile_pool(name="epool", bufs=16) as epool, \
                 tc.tile_pool(name="mpool", bufs=1) as mpool, \
                 tc.tile_pool(name="aop", bufs=4) as aop, \
                 tc.tile_pool(name="zpool", bufs=2) as zpool, \
                 tc.tile_pool(name="ps_s", bufs=3, space="PSUM") as ps_s, \
                 tc.tile_pool(name="ps_o", bufs=2, space="PSUM") as ps_o, \
                 tc.tile_pool(name="ps_z", bufs=2, space="PSUM") as ps_z, \
                 tc.tile_pool(name="ps_x", bufs=1, space="PSUM") as ps_x:
                vnats, mtile_maps = {}, {}

                def emit_vnat(b):
                    boff = b * S
                    vnat = vnatp.tile([128, S], BF16, name=f"vnat{b}",
                                      tag="vnat")
                    for kt in range(SKT):
                        ptr = ps_x.tile([128, 128], BF16, name=f"ptr{b}_{kt}",
                                        tag="ptr")
                        nc.tensor.transpose(
                            ptr[:],
                            vT_sb[:, boff + kt * 128:boff + (kt + 1) * 128],
                            idn[:])
                        nc.scalar.copy(vnat[:, kt * 128:(kt + 1) * 128],
                                       ptr[:])
                    vnats[b] = vnat

                def emit_masks(b, qc):
                    mtiles = {}
                    for kt in range(SKT):
                        if status[b, kt, qc] == 2:
                            mi = index[(b, kt, qc)]
                            mt_ = mpool.tile([128, 512], F32,
                                             name=f"mt{b}{qc}{kt}",
                                             tag=f"mt{kt}")
                            nc.sync.dma_start(
                                out=mt_[:],
                                in_=maskP[mi * 128:(mi + 1) * 128, :])
                            mtiles[kt] = mt_
                    mtile_maps[(b, qc)] = mtiles

                def emit_scores(g):
                    b, qc, h = g["b"], g["qc"], g["h"]
                    boff = b * S
                    mtiles = mtile_maps[(b, qc)]
                    qsl = qT_sb[h][:, boff + qc * 512:boff + (qc + 1) * 512]
                    for kt in g["kts"]:
                        ps_ = ps_s.tile([128, 512], F32,
                                        name=f"s{b}{h}{qc}{kt}", tag="ps")
                        nc.tensor.matmul(
                            ps_[:],
                            kT_sb[:, boff + kt * 128:boff + (kt + 1) * 128],
                            qsl, start=True, stop=True,
                            skip_group_check=True)
                        e = epool.tile([128, 512], BF16,
                                       name=f"e{b}{h}{qc}{kt}", tag="e")
                        if kt in mtiles:
                            # scores*sqk + mask, then exp
                            nc.vector.scalar_tensor_tensor(
                                ps_[:], ps_[:], BSQK, mtiles[kt][:],
                                ALU.mult, ALU.add)
                            nc.scalar.activation(e[:], ps_[:], ACTF.Exp)
                        else:
                            nc.scalar.activation(e[:], ps_[:], ACTF.Exp,
                                                 scale=BSQK)
                        g["es"].append((kt, e))

                def emit_drains(g):
                    b, qc, h, ch = g["b"], g["qc"], g["h"], g["ch"]
                    vnat = vnats[b]
                    pz = ps_z.tile([1, 512], F32, name=f"pz{b}{h}{qc}",
                                   tag="pz")
                    po = ps_o.tile([128, 512], F32, name=f"po{b}{h}{qc}",
                                   tag="po")
                    n = len(g["es"])
                    for j, (kt, e) in enumerate(g["es"]):
                        fst, lst = (j == 0), (j == n - 1)
                        nc.tensor.matmul(pz[:], onesb[:], e[:], start=fst,
                                         stop=lst, skip_group_check=True)
                        nc.tensor.matmul(
                            po[:], vnat[:, kt * 128:(kt + 1) * 128], e[:],
                            start=fst, stop=lst, skip_group_check=True)
                    # zr = gamma_v / z, broadcast to 128 partitions
                    zr = zpool.tile([1, 512], F32, name=f"zr{b}{h}{qc}",
                                    tag="zr")
                    nc.vector.reciprocal(zr[:], pz[:])
                    nc.vector.tensor_scalar(zr[:], zr[:], grow[0:1, 4:5],
                                            None, ALU.mult)
                    zb = zpool.tile([128, 512], F32, name=f"zb{b}{h}{qc}",
                                    tag="zb")
                    nc.gpsimd.partition_broadcast(zb[:], zr[:])
                    ao = aop.tile([128, 512], BF16, name=f"ao{b}{h}{qc}",
                                  tag="ao")
                    nc.vector.tensor_mul(ao[:], po[:], zb[:])
                    nc.sync.dma_start(
                        out=agin[ch][h * 128:(h + 1) * 128, :], in_=ao[:])
                    if h == QH - 1:
                        nc.gpsimd.collective_compute(
                            "AllGather", ALU.bypass, replica_groups=RG,
                            ins=[agin[ch][:].opt()],
                            outs=[agout[ch][:].opt()])

                groups = []
                for b in range(B):
                    for qc in range(SQC):
                        kts = [kt for kt in range(SKT)
                               if status[b, kt, qc] != 1]
                        assert kts, "fully-masked softmax row"
                        for h in range(QH):
                            groups.append({"b": b, "qc": qc, "h": h,
                                           "ch": b * SQC + qc, "kts": kts,
                                           "es": []})
                prev = None
                for g in groups:
                    if g["h"] == 0 and g["qc"] == 0:
                        emit_vnat(g["b"])
                    if g["h"] == 0:
                        emit_masks(g["b"], g["qc"])
                    emit_scores(g)
                    if prev is not None:
                        emit_drains(prev)
                    prev = g
                emit_drains(prev)

            # ---- phase 3: o_proj in bf16 (out in natural [token, m]) ----
            with tc.tile_pool(name="a3", bufs=1) as a3, \
                 tc.tile_pool(name="o3", bufs=2) as o3, \
                 tc.tile_pool(name="p3", bufs=2, space="PSUM") as p3:
                for ch in range(NTC):
                    ats = []
                    for kt in range(NKT):
                        at = a3.tile([128, 512], BF16, name=f"at{ch}_{kt}",
                                     tag=f"at{kt}")
                        nc.sync.dma_start(
                            out=at[:],
                            in_=agout[ch][kt * 128:(kt + 1) * 128, :])
                        ats.append(at)
                    for tt in range(4):
                        pout = p3.tile([128, 512], F32, name=f"po3_{ch}{tt}",
                                       tag="pout")
                        for kt in range(NKT):
                            nc.tensor.matmul(
                                pout[:],
                                ats[kt][:, tt * 128:(tt + 1) * 128],
                                wo_sb[:, kt * MSH:(kt + 1) * MSH],
                                start=(kt == 0), stop=(kt == NKT - 1),
                                skip_group_check=True)
                        osb = o3.tile([128, 512], F32, name=f"osb{ch}{tt}",
                                      tag="osb")
                        nc.scalar.activation(osb[:], pout[:], ACTF.Copy,
                                             scale=BGO)
                        nc.sync.dma_start(
                            out=outN[ch * 512 + tt * 128:
                                     ch * 512 + (tt + 1) * 128, :],
                            in_=osb[:])
            w3.release()
            qkvp.release()

    nc.compile()
    return nc


def _pretile(wshard):
    """[OUT, H] torch-layout shard -> [128, NKT*OUT] tiled transpose."""
    out_dim = wshard.shape[0]
    wt = np.ascontiguousarray(wshard.T)               # (H, out)
    return np.ascontiguousarray(
        wt.reshape(NKT, 128, out_dim).transpose(1, 0, 2).reshape(
            128, NKT * out_dim))


def kernel(hidden_states, Wq, Wk, Wv, Wo, attention_mask, position_ids):
    from concourse.bass_utils import run_bass_kernel_spmd
    from concourse.bass_interp import get_hw_module

    hs = np.ascontiguousarray(np.asarray(hidden_states, dtype=np.float32))
    Wq = np.asarray(Wq, dtype=np.float32)
    Wk = np.asarray(Wk, dtype=np.float32)
    Wv = np.asarray(Wv, dtype=np.float32)
    Wo = np.asarray(Wo, dtype=np.float32)
    mask = np.asarray(attention_mask, dtype=np.float32)
    posf = np.ascontiguousarray(
        np.asarray(position_ids).reshape(1, T).astype(np.float32))

    status, index, packed = _classify_mask(mask)
    n_packed = packed.shape[0] // 128

    key = (status.tobytes(), n_packed)
    if key not in _cache:
        nc = _build(status, index, n_packed)
        nc.m = get_hw_module(nc.m)
        _cache[key] = nc
    nc = _cache[key]

    import ml_dtypes
    xT = np.ascontiguousarray(
        hs.reshape(T, H).T.astype(ml_dtypes.bfloat16))
    in_maps = []
    for c in range(NCORES):
        in_maps.append({
            "xT": xT,
            "wqt": _pretile(Wq[c * QH * HD:(c + 1) * QH * HD, :]),
            "wkt": _pretile(Wk[c * HD:(c + 1) * HD, :]),
            "wvt": _pretile(Wv[c * HD:(c + 1) * HD, :]),
            "wot": _pretile(Wo[c * MSH:(c + 1) * MSH, :]),
            "maskP": packed,
            "pos": posf,
        })
    res = run_bass_kernel_spmd(nc, in_maps, core_ids=list(range(NCORES)),
                               trace=bool(os.environ.get("BITNET_TRACE")))
    global last_exec_time_ns
    last_exec_time_ns = res.exec_time_ns
    out = np.concatenate(
        [res.results[c]["outN"] for c in range(NCORES)], axis=1)  # (T, H)
    return np.ascontiguousarray(out).reshape(B, S, H).astype(np.float32)
